# revision 1
# baseline (speedup 1.0000x reference)
"""Trainium2 8-core Bass kernel for the Adapted_complementor GNN (2-layer GAT).

Strategy (graph/data parallel per sharding hint):
  - nodes sharded contiguously: core c owns [NL*c, NL*(c+1)), padded to NLP rows
  - dense phase per core: x_o/x_u/scores/message_u via TensorE matmuls on a
    host-pre-transposed x shard; deltaX via tiny AllReduce
  - per layer: bf16 "table" row per node [msg(128bf16)|4 scores f32|pad] = 512B,
    AllGather -> every core holds the full table; edges routed by dst on host,
    one 512B dma_gather per edge slot fetches msg+score together
  - segment softmax without max-subtraction (value range is tiny; exact in f32)
  - per-dst-tile [128 dst x K] layout; a_tar is a per-partition scalar; pads get
    a -1e4 additive bias (exp -> 0); 1/Z folded into the final per-tile scale
  - int16 gather-index limit: table split into two 25088-row halves, per-half
    slot streams per tile
  - layer-1 output blended with (1-m) and scatter-added into the out rows
"""

import os
import sys
import numpy as np

for p in ("/opt/trn_rl_repo", "/root/.axon_site/_ro/trn_rl_repo"):
    if os.path.isdir(p) and p not in sys.path:
        sys.path.insert(0, p)

import ml_dtypes  # noqa: E402  (ships with jax)

# ---------------------------------------------------------------- constants
N_CORES = 8
N_NODES = int(os.environ.get("GNN_N", "50000"))
N_EDGES = int(os.environ.get("GNN_E", "800000"))
DIM_O = 256
DIM_U = 256
HID = 128
NEG_SLOPE = 0.1

NL = N_NODES // N_CORES            # 6250 real nodes per core
T = (NL + 127) // 128              # 49 dst tiles per core
NLP = T * 128                      # 6272 padded rows per core
NG = NLP * N_CORES                 # 50176 global table rows
HROWS = NLP // 2                   # 3136 local rows per half-table
HALF = HROWS * N_CORES             # 25088 rows per half (< 32768: int16-safe)
TBL_COLS = 256                     # bf16 cols per table row (512B)
PAD_BIAS = -1.0e4
Z_EPS = 1e-30
GROUP_COL_BUDGET = int(os.environ.get("GNN_GCB", "96"))             # max summed K-cols (both halves) per gather group

F32 = None  # filled after bass import
BF16 = None
I16 = None


def _wrap16(flat):
    """Pack a flat int16 index list into the [128, n/16] SBUF wrap layout
    (16-partition wrap, replicated 8x across the partition groups)."""
    flat = np.asarray(flat, dtype=np.int16)
    n = len(flat)
    assert n % 16 == 0, n
    w = np.ascontiguousarray(flat.reshape(n // 16, 16).T)
    return np.tile(w, (8, 1))


# ---------------------------------------------------------------- host routing
def _route_layer(src_rows, dst, n_cores=N_CORES):
    """Route one layer's edges.

    src_rows: global table-row id of each edge's source (w.r.t. the table the
              layer gathers from). dst: global node id of each edge's dest.
    Returns (sched, per_core) where sched has the SPMD-shared tile schedule and
    per_core the per-core index/bias arrays.
    """
    dst_core = dst // NL
    dst_loc = dst % NL
    # src_rows: (core, local) pairs encoded as core*NLP+local; re-encode into
    # half-tables split by local row (< HROWS -> A, else B)
    s_core = src_rows // NLP
    s_loc = src_rows % NLP
    in_b = s_loc >= HROWS
    src_rows = np.where(in_b, s_core * HROWS + (s_loc - HROWS) + HALF,
                        s_core * HROWS + s_loc)

    per_core_raw = []
    K0 = np.zeros((n_cores, T), np.int64)
    K1 = np.zeros((n_cores, T), np.int64)
    for c in range(n_cores):
        sel = np.nonzero(dst_core == c)[0]
        ld = dst_loc[sel]
        rs = src_rows[sel]
        half = (rs >= HALF).astype(np.int64)
        c0 = np.bincount(ld[half == 0], minlength=NL)
        c1 = np.bincount(ld[half == 1], minlength=NL)
        # descending lexsort by c0, snake on c1 within equal-c0 runs:
        # keeps both per-tile maxima tight
        order = np.lexsort(((-1) ** (c0 % 2) * -c1, -c0))
        sortpos = np.empty(NL, np.int64)
        sortpos[order] = np.arange(NL)
        sp = sortpos[ld]
        for t in range(T):
            lo, hi = t * 128, min((t + 1) * 128, NL)
            K0[c, t] = c0[order[lo:hi]].max(initial=0)
            K1[c, t] = c1[order[lo:hi]].max(initial=0)
        per_core_raw.append((sel, ld, rs, half, order, sortpos, sp))

    K0s = K0.max(axis=0)
    K1s = K1.max(axis=0)
    coff0 = np.concatenate([[0], np.cumsum(K0s)])
    coff1 = np.concatenate([[0], np.cumsum(K1s)])
    C0, C1 = int(coff0[-1]), int(coff1[-1])

    # greedy tile grouping under the SBUF column budget
    groups = []
    cur, cost = [], 0
    for t in range(T):
        w = int(K0s[t] + K1s[t])
        if cur and (cost + w > GROUP_COL_BUDGET or len(cur) >= 8):
            groups.append(cur)
            cur, cost = [], 0
        cur.append(t)
        cost += w
    if cur:
        groups.append(cur)

    sched = dict(K0=K0s, K1=K1s, coff0=coff0, coff1=coff1, C0=C0, C1=C1,
                 groups=groups)

    per_core = []
    for c in range(n_cores):
        sel, ld, rs, half, order, sortpos, sp = per_core_raw[c]
        t_of = sp // 128
        d_of = sp % 128
        # rank of each edge within its (dst, half) group
        ek = np.lexsort((rs, half, sp))      # group edges by (sp, half)
        key = sp[ek] * 2 + half[ek]
        grp_start = np.concatenate([[0], np.nonzero(np.diff(key))[0] + 1])
        gid = np.zeros(len(ek), np.int64)
        gid[grp_start[1:]] = 1
        gid = np.cumsum(gid)
        rank = np.arange(len(ek)) - grp_start[gid]
        kk = np.empty(len(ek), np.int64)
        kk[ek] = rank

        idx0 = np.zeros(C0 * 128, np.int16)
        idx1 = np.zeros(C1 * 128, np.int16)
        pb0 = np.full((128, C0), PAD_BIAS, np.float32)
        pb1 = np.full((128, C1), PAD_BIAS, np.float32)
        h0 = half == 0
        col0 = coff0[t_of[h0]] + kk[h0]
        idx0[col0 * 128 + d_of[h0]] = rs[h0].astype(np.int16)
        pb0[d_of[h0], col0] = 0.0
        h1 = half == 1
        col1 = coff1[t_of[h1]] + kk[h1]
        idx1[col1 * 128 + d_of[h1]] = (rs[h1] - HALF).astype(np.int16)
        pb1[d_of[h1], col1] = 0.0

        per_core.append(dict(order=order, sortpos=sortpos,
                             idx0=_wrap16(idx0), idx1=_wrap16(idx1),
                             pb0=pb0, pb1=pb1))
    return sched, per_core


def _d_idx(order):
    """[16, NLP/16] wrap of the local row ids to D-gather, -1 trailing pad."""
    flat = np.full(NLP, -1, np.int16)
    flat[:NL] = order.astype(np.int16)
    return _wrap16(flat)


# ---------------------------------------------------------------- bass builder
def build_graph(s0, s1):
    """Build the SPMD bass graph given the two layers' shared schedules."""
    from concourse import bass, bacc, tile
    from concourse import mybir

    global F32, BF16, I16
    F32 = mybir.dt.float32
    BF16 = mybir.dt.bfloat16
    I16 = mybir.dt.int16
    AF = mybir.ActivationFunctionType
    OP = mybir.AluOpType

    nc = bacc.Bacc("TRN2", target_bir_lowering=False, debug=False,
                   num_devices=N_CORES,
                   dynamic_dma_scratch_size=int(os.environ.get("GNN_SCR", "16384")))

    # ---------------- parameters (per-core shards prepared on host)
    xT = nc.declare_dram_parameter("xT", [512, NLP], I16, isOutput=False)  # bf16 bits
    m_wrap = nc.declare_dram_parameter("m_wrap", [128, T], F32, isOutput=False)
    v_wrap = nc.declare_dram_parameter("v_wrap", [128, T], F32, isOutput=False)
    W_o = nc.declare_dram_parameter("W_o", [256, HID], F32, isOutput=False)
    W_u = nc.declare_dram_parameter("W_u", [256, HID], F32, isOutput=False)
    w4 = nc.declare_dram_parameter("w4", [HID, 4], F32, isOutput=False)
    W_diff = nc.declare_dram_parameter("W_diff", [HID, HID], F32, isOutput=False)
    b_diff = nc.declare_dram_parameter("b_diff", [1, HID], F32, isOutput=False)
    W_g = nc.declare_dram_parameter("W_g", [2 * HID, HID], F32, isOutput=False)
    ident = nc.declare_dram_parameter("ident", [128, 128], F32, isOutput=False)
    mask16 = nc.declare_dram_parameter("mask16", [128, 128], I16, isOutput=False)

    idx0_l0 = nc.declare_dram_parameter("idx0_l0", [128, s0["C0"] * 8], I16, False)
    idx1_l0 = nc.declare_dram_parameter("idx1_l0", [128, s0["C1"] * 8], I16, False)
    pb0_l0 = nc.declare_dram_parameter("pb0_l0", [128, s0["C0"]], F32, False)
    pb1_l0 = nc.declare_dram_parameter("pb1_l0", [128, s0["C1"]], F32, False)
    idxd_l0 = nc.declare_dram_parameter("idxd_l0", [128, NLP // 16], I16, False)
    idx0_l1 = nc.declare_dram_parameter("idx0_l1", [128, s1["C0"] * 8], I16, False)
    idx1_l1 = nc.declare_dram_parameter("idx1_l1", [128, s1["C1"] * 8], I16, False)
    pb0_l1 = nc.declare_dram_parameter("pb0_l1", [128, s1["C0"]], F32, False)
    pb1_l1 = nc.declare_dram_parameter("pb1_l1", [128, s1["C1"]], F32, False)
    idxd_l1 = nc.declare_dram_parameter("idxd_l1", [128, NLP // 16], I16, False)
    scat_l1 = nc.declare_dram_parameter("scat_l1", [128, NLP // 16], I16, False)
    oms_l1 = nc.declare_dram_parameter("oms_l1", [128, T], F32, False)

    outp = nc.declare_dram_parameter("out", [NLP, 2 * HID], F32, isOutput=True)

    # ---------------- internal DRAM
    t0loc = nc.dram_tensor("t0loc", [NLP, TBL_COLS], BF16)
    t1loc = nc.dram_tensor("t1loc", [NLP, TBL_COLS], BF16)
    t0fa = nc.dram_tensor("t0fa", [HALF, TBL_COLS], BF16, addr_space="Shared")
    t0fb = nc.dram_tensor("t0fb", [HALF, TBL_COLS], BF16, addr_space="Shared")
    t1fa = nc.dram_tensor("t1fa", [HALF, TBL_COLS], BF16, addr_space="Shared")
    t1fb = nc.dram_tensor("t1fb", [HALF, TBL_COLS], BF16, addr_space="Shared")
    arin = nc.dram_tensor("arin", [2, 132], F32)
    arout = nc.dram_tensor("arout", [2, 132], F32, addr_space="Shared")

    RG = [list(range(N_CORES))]

    from concourse import library_config

    with tile.TileContext(nc) as tc:
        nc.gpsimd.load_library(library_config.mlp)
        with tc.tile_pool(name="persist", bufs=1) as pp:
            # persistent small tensors
            id_s = pp.tile([128, 128], F32)
            nc.sync.dma_start(id_s[:], ident[:])
            w4s = pp.tile([HID, 4], F32)
            nc.sync.dma_start(w4s[:], w4[:])
            wd = pp.tile([HID, HID], F32)
            nc.sync.dma_start(wd[:], W_diff[:])
            bd = pp.tile([1, HID], F32)
            nc.sync.dma_start(bd[:], b_diff[:])
            wgt = pp.tile([128, HID], F32)
            nc.sync.dma_start(wgt[:], W_g[0:128, :])
            wgb = pp.tile([128, HID], F32)
            nc.sync.dma_start(wgb[:], W_g[128:256, :])
            m_s = pp.tile([128, T], F32)
            nc.sync.dma_start(m_s[:], m_wrap[:])
            v_s = pp.tile([128, T], F32)
            nc.sync.dma_start(v_s[:], v_wrap[:])
            ones_col = pp.tile([128, 1], F32)
            nc.vector.memset(ones_col[:], 1.0)
            ones_row = pp.tile([1, 128], F32)
            nc.vector.memset(ones_row[:], 1.0)
            ones11 = pp.tile([1, 1], F32)
            nc.vector.memset(ones11[:], 1.0)
            negm = pp.tile([128, T], F32)
            nc.vector.tensor_scalar_mul(negm[:], m_s[:], -1.0)
            om_s = pp.tile([128, T], F32)     # (1-m)*valid = valid - m
            nc.vector.tensor_sub(om_s[:], v_s[:], m_s[:])
            mm2 = pp.tile([128, T, 2], F32)   # lhsT slices for deltaX partials
            nc.vector.tensor_copy(mm2[:, :, 0], m_s[:])
            nc.vector.tensor_copy(mm2[:, :, 1], om_s[:])

            # prebuilt matmul RHS: RA_c = [W_o chunk | score vec cols], RU, WuG
            with tc.tile_pool(name="dense", bufs=1) as dp, \
                 tc.tile_pool(name="densepsum", bufs=2, space="PSUM") as pmm, \
                 tc.tile_pool(name="psum1", bufs=1, space="PSUM") as p1, \
                 tc.tile_pool(name="xtp", bufs=1) as xp:

                # bf16 weights: load f32, cast once
                RAf = [dp.tile([128, HID], F32, name=f"RAf{c}") for c in range(2)]
                RUf = [dp.tile([128, HID], F32, name=f"RUf{c}") for c in range(2)]
                RA = [dp.tile([128, HID + 4], BF16, name=f"RA{c}") for c in range(2)]
                RU = [dp.tile([128, HID], BF16, name=f"RU{c}") for c in range(2)]
                WuG = [dp.tile([128, HID], BF16, name=f"WuG{c}") for c in range(2)]
                wgt16 = dp.tile([128, HID], BF16, tag="wgt16")
                nc.vector.tensor_copy(wgt16[:], wgt[:])
                w4s16 = dp.tile([HID, 4], BF16, tag="w4s16")
                nc.vector.tensor_copy(w4s16[:], w4s[:])
                for c in range(2):
                    nc.sync.dma_start(RAf[c][:], W_o[128 * c:128 * (c + 1), :])
                    nc.sync.dma_start(RUf[c][:], W_u[128 * c:128 * (c + 1), :])
                    nc.vector.tensor_copy(RA[c][:, 0:HID], RAf[c][:])
                    nc.vector.tensor_copy(RU[c][:], RUf[c][:])
                # wv = W_o @ w4 (via W_o^T chunks), WuG = W_u @ Wg_top
                for c in range(2):
                    ptp = pmm.tile([128, 128], F32, tag="a")
                    nc.tensor.transpose(ptp[:], RAf[c][:], id_s[:])
                    wt = dp.tile([128, 128], BF16, tag="wt")
                    nc.vector.tensor_copy(wt[:], ptp[:])
                    pwv = pmm.tile([128, 4], F32, tag="b")
                    nc.tensor.matmul(pwv[:], wt[:], w4s16[:], start=True, stop=True)
                    nc.vector.tensor_copy(RA[c][:, HID:HID + 4], pwv[:])

                    ptp2 = pmm.tile([128, 128], F32, tag="a")
                    nc.tensor.transpose(ptp2[:], RUf[c][:], id_s[:])
                    wt2 = dp.tile([128, 128], BF16, tag="wt")
                    nc.vector.tensor_copy(wt2[:], ptp2[:])
                    pwg = pmm.tile([128, 128], F32, tag="b")
                    nc.tensor.matmul(pwg[:], wt2[:], wgt16[:], start=True, stop=True)
                    nc.vector.tensor_copy(WuG[c][:], pwg[:])

                # resident transposed x shard, bf16: 4 chunks of [128, NLP]
                xTc = [xp.tile([128, NLP], BF16, name=f"xTc{c}") for c in range(4)]
                for c in range(4):
                    nc.sync.dma_start(xTc[c][:], xT[128 * c:128 * (c + 1), :].bitcast(BF16))

                xo_s = dp.tile([128, T, HID + 4], F32, tag="xo_s")
                xu_s = dp.tile([128, T, HID], F32, tag="xu_s")
                pdx = p1.tile([2, 128], F32, tag="pdx")
                pcnt = p1.tile([2, 1], F32, tag="pcnt")

                # ---- dense pass A
                for t in range(T):
                    sl = slice(128 * t, 128 * (t + 1))
                    po = pmm.tile([128, HID + 4], F32, tag="a")
                    nc.tensor.matmul(po[:], xTc[0][:, sl], RA[0][:],
                                     start=True, stop=False)
                    nc.tensor.matmul(po[:], xTc[1][:, sl], RA[1][:],
                                     start=False, stop=True)
                    pu = pmm.tile([128, HID], F32, tag="b")
                    nc.tensor.matmul(pu[:], xTc[2][:, sl], RU[0][:],
                                     start=True, stop=False)
                    nc.tensor.matmul(pu[:], xTc[3][:, sl], RU[1][:],
                                     start=False, stop=True)
                    nc.scalar.copy(xo_s[:, t, :], po[:])
                    nc.vector.tensor_copy(xu_s[:, t, :], pu[:])
                    t1t = dp.tile([128, HID], F32, tag="t1t", bufs=3)
                    nc.vector.tensor_scalar_mul(t1t[:], xu_s[:, t, :],
                                                m_s[:, t:t + 1])
                    # deltaX partials: [m, valid-m]^T @ [x_o], and counts
                    nc.tensor.matmul(pdx[:], mm2[:, t, :], xo_s[:, t, 0:HID],
                                     start=(t == 0), stop=(t == T - 1))
                    nc.tensor.matmul(pcnt[:], mm2[:, t, :], ones_col[:],
                                     start=(t == 0), stop=(t == T - 1))
                    # output cols 0:128 = x_o ; cols 128:256 = x_u*m (+= xuhat later)
                    nc.sync.dma_start(outp[sl, 0:HID], xo_s[:, t, 0:HID])
                    nc.sync.dma_start(outp[sl, HID:2 * HID], t1t[:])

                # ---- deltaX AllReduce
                ar_s = dp.tile([2, 132], F32, tag="ar_s")
                nc.vector.memset(ar_s[:], 0.0)
                nc.vector.tensor_copy(ar_s[:, 0:128], pdx[:])
                nc.vector.tensor_copy(ar_s[:, 128:129], pcnt[:])
                nc.sync.dma_start(arin[:], ar_s[:])
                nc.gpsimd.collective_compute(
                    "AllReduce", OP.add, ins=[arin[:]], outs=[arout[:]],
                    replica_groups=RG)
                ars = dp.tile([2, 132], F32, tag="ars")
                nc.sync.dma_start(ars[:], arout[:])
                rec = dp.tile([2, 1], F32, tag="rec")
                nc.vector.reciprocal(rec[:], ars[:, 128:129])
                means = dp.tile([2, 128], F32, tag="means")
                nc.vector.tensor_scalar_mul(means[:], ars[:, 0:128], rec[:])
                ptp3 = pmm.tile([128, 128], F32, tag="a")
                nc.tensor.transpose(ptp3[:, 0:2], means[:], id_s[0:2, 0:2])
                mT = dp.tile([128, 2], F32, tag="mT")
                nc.vector.tensor_copy(mT[:], ptp3[:, 0:2])
                dxc = dp.tile([128, 1], F32, tag="dxc")
                nc.vector.tensor_sub(dxc[:], mT[:, 0:1], mT[:, 1:2])
                pad_ = pmm.tile([1, 128], F32, tag="b")
                nc.tensor.matmul(pad_[:], dxc[:], wd[:], start=True, stop=True)
                adr = dp.tile([1, 128], F32, tag="adr")
                nc.vector.tensor_add(adr[:], pad_[:], bd[:])
                pac = pmm.tile([128, 1], F32, tag="a")
                nc.tensor.matmul(pac[:], adr[:], ones11[:], start=True, stop=True)
                adc = dp.tile([128, 1], F32, tag="adc")
                nc.vector.tensor_copy(adc[:], pac[:])
                pcr = pmm.tile([1, 128], F32, tag="b")
                nc.tensor.matmul(pcr[:], adc[:], wgb[:], start=True, stop=True)
                crow = dp.tile([1, 128], F32, tag="crow")
                nc.vector.tensor_copy(crow[:], pcr[:])

                # ---- dense pass B: message_u + scores -> full t0loc rows
                t0st = dp.tile([128, T, TBL_COLS], BF16, tag="t0st")
                nc.vector.memset(t0st[:], 0.0)
                nc.vector.tensor_copy(
                    t0st[:, :, 128:136].bitcast(F32),
                    xo_s[:, :, HID:HID + 4])
                for t in range(T):
                    sl = slice(128 * t, 128 * (t + 1))
                    pD = pmm.tile([128, HID], F32, tag="a")
                    nc.tensor.matmul(pD[:], xTc[2][:, sl], WuG[0][:],
                                     start=True, stop=False)
                    nc.tensor.matmul(pD[:], xTc[3][:, sl], WuG[1][:],
                                     start=False, stop=False)
                    nc.tensor.matmul(pD[:], ones_row[:], crow[:],
                                     start=False, stop=True)
                    nc.vector.scalar_tensor_tensor(
                        t0st[:, t, 0:HID], pD[:], negm[:, t:t + 1], xu_s[:, t, :],
                        op0=OP.mult, op1=OP.add)
                    # 4 f32 scores bit-stored in bf16 cols 128:136


                t0dst = bass.AP(t0loc, 0, [[TBL_COLS, 128], [128 * TBL_COLS, T],
                                           [1, TBL_COLS]])
                nc.sync.dma_start(t0dst, t0st[:])

            # ---------------- AllGather table0; edge layers
            phase = os.environ.get("GNN_PHASE", "full")
            if phase != "dense":
                nc.gpsimd.collective_compute(
                    "AllGather", OP.bypass, ins=[t0loc[0:HROWS, :]],
                    outs=[t0fa[:]], replica_groups=RG)
                nc.gpsimd.collective_compute(
                    "AllGather", OP.bypass, ins=[t0loc[HROWS:NLP, :]],
                    outs=[t0fb[:]], replica_groups=RG)

            if phase not in ("dense", "ag"):
                _edge_layer(nc, tc, 0, s0, (t0fa, t0fb), t0loc, t1loc,
                            idx0_l0, idx1_l0, pb0_l0, pb1_l0, idxd_l0,
                            None, None, mask16, outp)

            if phase == "full":
                nc.gpsimd.collective_compute(
                    "AllGather", OP.bypass, ins=[t1loc[0:HROWS, :]],
                    outs=[t1fa[:]], replica_groups=RG)
                nc.gpsimd.collective_compute(
                    "AllGather", OP.bypass, ins=[t1loc[HROWS:NLP, :]],
                    outs=[t1fb[:]], replica_groups=RG)

                _edge_layer(nc, tc, 1, s1, (t1fa, t1fb), t1loc, None,
                            idx0_l1, idx1_l1, pb0_l1, pb1_l1, idxd_l1,
                            scat_l1, oms_l1, mask16, outp)

    nc.finalize()
    return nc


def _edge_layer(nc, tc, layer, sched, tfull, tloc, tnext,
                idx0p, idx1p, pb0p, pb1p, idxdp, scatp, omsp, mask16p, outp):
    from concourse import bass, mybir
    OP = mybir.AluOpType
    AF = mybir.ActivationFunctionType
    F32 = mybir.dt.float32
    BF16 = mybir.dt.bfloat16
    I16 = mybir.dt.int16

    K0, K1 = sched["K0"], sched["K1"]
    coff0, coff1 = sched["coff0"], sched["coff1"]
    C0, C1 = sched["C0"], sched["C1"]
    groups = sched["groups"]
    g0max = max(int(sum(K0[t] for t in g)) for g in groups)
    g1max = max(int(sum(K1[t] for t in g)) for g in groups)

    with tc.tile_pool(name=f"edge{layer}", bufs=1) as ep, \
         tc.tile_pool(name=f"gath{layer}", bufs=2) as gp, \
         tc.tile_pool(name=f"work{layer}", bufs=3) as wp, \
         tc.tile_pool(name=f"epsum{layer}", bufs=1, space="PSUM") as pp2:

        idx0 = ep.tile([128, C0 * 8], I16)
        nc.sync.dma_start(idx0[:], idx0p[:])
        idx1 = ep.tile([128, C1 * 8], I16)
        nc.sync.dma_start(idx1[:], idx1p[:])
        pb0 = ep.tile([128, C0], F32)
        nc.sync.dma_start(pb0[:], pb0p[:])
        pb1 = ep.tile([128, C1], F32)
        nc.sync.dma_start(pb1[:], pb1p[:])
        idxd = ep.tile([128, NLP // 16], I16)
        nc.sync.dma_start(idxd[:], idxdp[:])
        mask = ep.tile([128, 128], BF16)
        nc.sync.dma_start(mask[:], mask16p[:].bitcast(BF16))
        if layer == 1:
            scat = ep.tile([128, NLP // 16], I16)
            nc.sync.dma_start(scat[:], scatp[:])
            oms = ep.tile([128, T], F32)
            nc.sync.dma_start(oms[:], omsp[:])

        # gather all dst rows once: a_tar (+ next-layer scores on layer 0)
        # (dma_gather calls are capped at 1024 indices: larger calls overflow
        #  the Q7-local index scratch and hard-crash the device)
        D = ep.tile([128, T, TBL_COLS], BF16)
        for c0 in range(0, T, 8):
            c1 = min(c0 + 8, T)
            ni = (c1 - c0) * 128
            nv = max(0, min(NL - c0 * 128, ni))
            nc.gpsimd.dma_gather(D[:, c0:c1, :], tloc[:],
                                 idxd[:, c0 * 8:c1 * 8], ni, nv, TBL_COLS)

        if layer == 0:
            msgst = ep.tile([128, T, TBL_COLS], BF16)
            nc.vector.memset(msgst[:], 0.0)
            # next-layer scores ride along in the D rows: one batched copy
            nc.vector.tensor_copy(msgst[:, :, 128:132], D[:, :, 132:136])

        for g in groups:
            gc00, gc01 = int(coff0[g[0]]), int(coff0[g[-1] + 1])
            gc10, gc11 = int(coff1[g[0]]), int(coff1[g[-1] + 1])
            n0, n1 = gc01 - gc00, gc11 - gc10
            G0 = gp.tile([128, g0max, TBL_COLS], BF16, tag="G0")
            G1 = gp.tile([128, g1max, TBL_COLS], BF16, tag="G1")
            tfa, tfb = tfull
            for cc0 in range(0, n0, 8):
                cc1 = min(cc0 + 8, n0)
                nc.gpsimd.dma_gather(
                    G0[:, cc0:cc1, :], tfa[:],
                    idx0[:, (gc00 + cc0) * 8:(gc00 + cc1) * 8],
                    (cc1 - cc0) * 128, (cc1 - cc0) * 128, TBL_COLS)
            for cc0 in range(0, n1, 8):
                cc1 = min(cc0 + 8, n1)
                nc.gpsimd.dma_gather(
                    G1[:, cc0:cc1, :], tfb[:],
                    idx1[:, (gc10 + cc0) * 8:(gc10 + cc1) * 8],
                    (cc1 - cc0) * 128, (cc1 - cc0) * 128, TBL_COLS)

            if layer == 1:
                scst = gp.tile([128, len(g), HID], F32, tag="scst")

            for ti, t in enumerate(g):
                k0, k1 = int(K0[t]), int(K1[t])
                o0, o1 = int(coff0[t]) - gc00, int(coff1[t]) - gc10
                atar = D[:, t, 130:132].bitcast(F32)
                z0 = wp.tile([128, 1], F32, tag="z0")
                z1 = wp.tile([128, 1], F32, tag="z1")
                ps = []
                for (h, kh, oh, G, pb, co) in (
                        (0, k0, o0, G0, pb0, int(coff0[t])),
                        (1, k1, o1, G1, pb1, int(coff1[t]))):
                    zh = (z0, z1)[h]
                    if kh == 0:
                        nc.vector.memset(zh[:], 0.0)
                        ps.append(None)
                        continue
                    sb = wp.tile([128, kh], F32, tag=f"sb{h}")
                    nc.vector.scalar_tensor_tensor(
                        sb[:], G[:, oh:oh + kh, 128:130].bitcast(F32)[:, :, 0],
                        atar, pb[:, co:co + kh], op0=OP.add, op1=OP.add)
                    nc.vector.scalar_tensor_tensor(
                        sb[:], sb[:], NEG_SLOPE, sb[:], op0=OP.mult, op1=OP.max)
                    ph = wp.tile([128, kh], F32, tag=f"p{h}")
                    nc.scalar.activation(ph[:], sb[:], AF.Exp, accum_out=zh[:])
                    ps.append(ph)
                z = wp.tile([128, 1], F32, tag="z")
                nc.vector.scalar_tensor_tensor(
                    z[:], z0[:], Z_EPS, z1[:], op0=OP.add, op1=OP.add)
                rz = wp.tile([128, 1], F32, tag="rz")
                nc.vector.reciprocal(rz[:], z[:])
                if layer == 1:
                    nc.vector.tensor_mul(rz[:], rz[:], oms[:, t:t + 1])
                # fold 1/Z (and (1-m) on layer 1) into the edge weights
                if ps[0] is not None:
                    nc.vector.tensor_scalar_mul(ps[0][:], ps[0][:], rz[:])
                if ps[1] is not None:
                    nc.vector.tensor_scalar_mul(ps[1][:], ps[1][:], rz[:])

                if layer == 0:
                    dest = msgst[:, t, 0:HID]
                else:
                    dest = scst[:, ti, :]
                chunks = ([(0, k) for k in range(k0)] +
                          [(1, k) for k in range(k1)])
                if not chunks:
                    if layer == 1:
                        nc.vector.memset(dest, 0.0)
                    continue
                pacc = pp2.tile([128, HID], F32, tag="pacc", bufs=2)
                for j, (h, k) in enumerate(chunks):
                    G, oh = (G0, o0) if h == 0 else (G1, o1)
                    ph = ps[h]
                    db = wp.tile([128, 128], BF16, tag="db", bufs=4)
                    nc.scalar.activation(db[:], mask[:], AF.Copy,
                                         scale=ph[:, k:k + 1])
                    nc.tensor.matmul(pacc[:], db[:], G[:, oh + k, 0:HID],
                                     start=(j == 0),
                                     stop=(j == len(chunks) - 1))
                nc.vector.tensor_copy(dest, pacc[:])

            if layer == 1:
                lo = g[0] * 128
                hi = min((g[-1] + 1) * 128, NL)
                nc.gpsimd.dma_scatter_add(
                    outp[:, HID:2 * HID], scst[:, 0:len(g), :],
                    scat[:, lo // 16:(g[-1] + 1) * 128 // 16],
                    len(g) * 128, hi - lo, HID, elem_step=2 * HID)

        if layer == 0:
            tA = HROWS // 128            # 24.5 -> 24 full tiles in half A
            d1 = bass.AP(tnext, 0,
                         [[TBL_COLS, 128], [128 * TBL_COLS, tA], [1, TBL_COLS]])
            nc.sync.dma_start(d1, msgst[:, 0:tA, :])
            d2 = bass.AP(tnext, tA * 128 * TBL_COLS,
                         [[TBL_COLS, 128], [128 * TBL_COLS, 1], [1, TBL_COLS]])
            nc.sync.dma_start(d2, msgst[:, tA:tA + 1, :])
            d3 = bass.AP(tnext, (tA + 1) * 128 * TBL_COLS,
                         [[TBL_COLS, 128], [128 * TBL_COLS, T - tA - 1],
                          [1, TBL_COLS]])
            nc.sync.dma_start(d3, msgst[:, tA + 1:T, :])


# ---------------------------------------------------------------- host driver
def _prep_inputs(x, central_mask, edge_index0, edge_index1,
                 W_o, W_u, w_src0, w_tar0, w_src1, w_tar1, W_diff, b_diff, W_g):
    """Shard + route everything; returns (s0, s1, in_maps)."""
    x = np.asarray(x, np.float32)
    m = np.asarray(central_mask, np.int32)
    e0 = np.asarray(edge_index0, np.int64)
    e1 = np.asarray(edge_index1, np.int64)

    # layer-0 table rows are node-ordered: r0(v) = (v//NL)*NLP + v%NL
    def r0(v):
        return (v // NL) * NLP + (v % NL)

    s0, pc0 = _route_layer(r0(e0[0]), e0[1])

    # layer-1 table rows are in each owner core's layer-0 sorted order
    sp0 = np.concatenate([pc0[c]["sortpos"] for c in range(N_CORES)])

    def r1(v):
        return (v // NL) * NLP + sp0[v]

    s1, pc1 = _route_layer(r1(e1[0]), e1[1])

    w4 = np.stack([np.asarray(w, np.float32) for w in
                   (w_src0, w_tar0, w_src1, w_tar1)], axis=1)
    ident = np.eye(128, dtype=np.float32)

    in_maps = []
    for c in range(N_CORES):
        xs = x[NL * c:NL * (c + 1)]
        xp = np.zeros((NLP, 512), np.float32)
        xp[:NL] = xs
        ms = np.zeros(NLP, np.float32)
        ms[:NL] = m[NL * c:NL * (c + 1)]
        vs = np.zeros(NLP, np.float32)
        vs[:NL] = 1.0
        o1 = pc1[c]["order"]
        omsv = np.zeros(NLP, np.float32)
        omsv[:NL] = 1.0 - ms[o1]
        scat_flat = np.full(NLP, -1, np.int16)
        scat_flat[:NL] = o1.astype(np.int16)
        # layer-1 D-gather reads t1loc rows = layer-0 sorted positions
        idxd1_flat = np.full(NLP, -1, np.int16)
        idxd1_flat[:NL] = pc0[c]["sortpos"][o1].astype(np.int16)

        in_maps.append({
            "xT": np.ascontiguousarray(xp.T).astype(
                ml_dtypes.bfloat16).view(np.int16),
            "m_wrap": np.ascontiguousarray(ms.reshape(T, 128).T),
            "v_wrap": np.ascontiguousarray(vs.reshape(T, 128).T),
            "W_o": np.asarray(W_o, np.float32),
            "W_u": np.asarray(W_u, np.float32),
            "w4": w4,
            "W_diff": np.asarray(W_diff, np.float32),
            "b_diff": np.asarray(b_diff, np.float32).reshape(1, HID),
            "W_g": np.asarray(W_g, np.float32),
            "ident": ident,
            "mask16": np.eye(128, dtype=ml_dtypes.bfloat16).view(np.int16),
            "idx0_l0": pc0[c]["idx0"], "idx1_l0": pc0[c]["idx1"],
            "pb0_l0": pc0[c]["pb0"], "pb1_l0": pc0[c]["pb1"],
            "idxd_l0": _d_idx(pc0[c]["order"]),
            "idx0_l1": pc1[c]["idx0"], "idx1_l1": pc1[c]["idx1"],
            "pb0_l1": pc1[c]["pb0"], "pb1_l1": pc1[c]["pb1"],
            "idxd_l1": _wrap16(idxd1_flat),
            "scat_l1": _wrap16(scat_flat),
            "oms_l1": np.ascontiguousarray(omsv.reshape(T, 128).T),
        })
    return s0, s1, in_maps


_CACHE = {}


def _install_ntff_hook():
    """Register the axon NTFF profiling hook if the image's antenv lacks it."""
    import types
    import contextlib
    try:
        from antenv.axon_hooks import get_axon_ntff_profile_hook  # noqa: F401
        return True
    except ImportError:
        pass
    try:
        if "/root/.axon_site" not in sys.path:
            sys.path.append("/root/.axon_site")
        from trn_agent_boot.trn_boot import _ntff_profile_via_ctypes
        import antenv
        hook = _ntff_profile_via_ctypes("/opt/axon/libaxon_pjrt.so")
        mod = types.ModuleType("antenv.axon_hooks")
        _h = [hook]
        mod.set_axon_ntff_profile_hook = lambda h: _h.__setitem__(0, h)
        mod.get_axon_ntff_profile_hook = lambda: _h[0]
        sys.modules["antenv.axon_hooks"] = mod
        antenv.axon_hooks = mod
        # artifact upload has no bucket in this container; stub it out
        from concourse import bass_utils as _bu
        _bu.upload_artifacts = lambda tmpdir: "local"
        return hook is not None
    except Exception as e:
        print("ntff hook install failed:", e)
        return False


def kernel(**inputs):
    s0, s1, in_maps = _prep_inputs(**inputs)

    from concourse.bass_utils import run_bass_kernel_spmd

    key = (tuple(s0["K0"]), tuple(s0["K1"]), tuple(s1["K0"]), tuple(s1["K1"]))
    if key not in _CACHE:
        _CACHE[key] = build_graph(s0, s1)
    nc = _CACHE[key]

    trace = bool(int(os.environ.get("GNN_TRACE", "0")))
    if trace:
        trace = _install_ntff_hook()
    res = run_bass_kernel_spmd(nc, in_maps, list(range(N_CORES)), trace=trace)
    if trace and res.exec_time_ns is not None:
        print(f"HW exec time: {res.exec_time_ns} ns")
        kernel.last_exec_ns = res.exec_time_ns
        kernel.last_profile = res.profile_json
    out = np.concatenate([res.results[c]["out"][:NL] for c in range(N_CORES)], 0)
    return out.astype(np.float32)


if __name__ == "__main__":
    import reference
    inp = {k: np.asarray(v) for k, v in reference.setup_inputs().items()}
    exp = np.asarray(reference.reference(**inp))
    act = kernel(**inp)
    err = np.abs(act - exp)
    rel = np.linalg.norm(act - exp) / np.linalg.norm(exp)
    print("max abs err:", err.max(), "rel:", rel)



# revision 8
# speedup vs baseline: 1.2852x; 1.2852x over previous
"""Trainium2 8-core Bass kernel for the Adapted_complementor GNN (2-layer GAT).

Strategy (graph/data parallel per sharding hint):
  - nodes sharded contiguously: core c owns [NL*c, NL*(c+1)), padded to NLP rows
  - dense phase per core: x_o/x_u/scores/message_u via TensorE matmuls on a
    host-pre-transposed x shard; deltaX via tiny AllReduce
  - per layer: bf16 "table" row per node [msg(128bf16)|4 scores f32|pad] = 512B,
    AllGather -> every core holds the full table; edges routed by dst on host,
    one 512B dma_gather per edge slot fetches msg+score together
  - segment softmax without max-subtraction (value range is tiny; exact in f32)
  - per-dst-tile [128 dst x K] layout; a_tar is a per-partition scalar; pads get
    a -1e4 additive bias (exp -> 0); 1/Z folded into the final per-tile scale
  - int16 gather-index limit: table split into two 25088-row halves, per-half
    slot streams per tile
  - layer-1 output blended with (1-m) and scatter-added into the out rows
"""

import os
import sys
import numpy as np

for p in ("/opt/trn_rl_repo", "/root/.axon_site/_ro/trn_rl_repo"):
    if os.path.isdir(p) and p not in sys.path:
        sys.path.insert(0, p)

import ml_dtypes  # noqa: E402  (ships with jax)

# ---------------------------------------------------------------- constants
N_CORES = 8
N_NODES = int(os.environ.get("GNN_N", "50000"))
N_EDGES = int(os.environ.get("GNN_E", "800000"))
DIM_O = 256
DIM_U = 256
HID = 128
NEG_SLOPE = 0.1

NL = N_NODES // N_CORES            # 6250 real nodes per core
T = (NL + 127) // 128              # 49 dst tiles per core
NLP = T * 128                      # 6272 padded rows per core
NG = NLP * N_CORES                 # 50176 global table rows
HROWS = NLP // 2                   # 3136 local rows per half-table
HALF = HROWS * N_CORES             # 25088 rows per half (< 32768: int16-safe)
TBL_COLS = 256                     # bf16 cols per table row (512B)
PAD_BIAS = -1.0e4
Z_EPS = 1e-30
GROUP_COL_BUDGET = int(os.environ.get("GNN_GCB", "96"))             # max summed K-cols (both halves) per gather group

F32 = None  # filled after bass import
BF16 = None
I16 = None

NSWQ = int(os.environ.get("GNN_NSWQ", "4"))
_QCTR = [0]


def _nq():
    """Round-robin SWDGE queue id for gather/scatter calls."""
    q = _QCTR[0] % NSWQ
    _QCTR[0] += 1
    return q


def _wrap16(flat):
    """Pack a flat int16 index list into the [128, n/16] SBUF wrap layout
    (16-partition wrap, replicated 8x across the partition groups)."""
    flat = np.asarray(flat, dtype=np.int16)
    n = len(flat)
    assert n % 16 == 0, n
    w = np.ascontiguousarray(flat.reshape(n // 16, 16).T)
    return np.tile(w, (8, 1))


# ---------------------------------------------------------------- host routing
def _route_layer(src_rows, dst, n_cores=N_CORES):
    """Route one layer's edges.

    src_rows: global table-row id of each edge's source (w.r.t. the table the
              layer gathers from). dst: global node id of each edge's dest.
    Returns (sched, per_core) where sched has the SPMD-shared tile schedule and
    per_core the per-core index/bias arrays.
    """
    dst_core = dst // NL
    dst_loc = dst % NL
    # src_rows: (core, local) pairs encoded as core*NLP+local; re-encode into
    # half-tables split by local row (< HROWS -> A, else B)
    s_core = src_rows // NLP
    s_loc = src_rows % NLP
    in_b = s_loc >= HROWS
    src_rows = np.where(in_b, s_core * HROWS + (s_loc - HROWS) + HALF,
                        s_core * HROWS + s_loc)

    per_core_raw = []
    K0 = np.zeros((n_cores, T), np.int64)
    K1 = np.zeros((n_cores, T), np.int64)
    for c in range(n_cores):
        sel = np.nonzero(dst_core == c)[0]
        ld = dst_loc[sel]
        rs = src_rows[sel]
        half = (rs >= HALF).astype(np.int64)
        c0 = np.bincount(ld[half == 0], minlength=NL)
        c1 = np.bincount(ld[half == 1], minlength=NL)
        # descending lexsort by c0, snake on c1 within equal-c0 runs:
        # keeps both per-tile maxima tight
        order = np.lexsort(((-1) ** (c0 % 2) * -c1, -c0))
        sortpos = np.empty(NL, np.int64)
        sortpos[order] = np.arange(NL)
        sp = sortpos[ld]
        for t in range(T):
            lo, hi = t * 128, min((t + 1) * 128, NL)
            K0[c, t] = c0[order[lo:hi]].max(initial=0)
            K1[c, t] = c1[order[lo:hi]].max(initial=0)
        per_core_raw.append((sel, ld, rs, half, order, sortpos, sp))

    K0s = K0.max(axis=0)
    K1s = K1.max(axis=0)
    coff0 = np.concatenate([[0], np.cumsum(K0s)])
    coff1 = np.concatenate([[0], np.cumsum(K1s)])
    C0, C1 = int(coff0[-1]), int(coff1[-1])

    # greedy tile grouping under the SBUF column budget
    groups = []
    cur, cost = [], 0
    for t in range(T):
        w = int(K0s[t] + K1s[t])
        if cur and (cost + w > GROUP_COL_BUDGET or len(cur) >= 8):
            groups.append(cur)
            cur, cost = [], 0
        cur.append(t)
        cost += w
    if cur:
        groups.append(cur)

    sched = dict(K0=K0s, K1=K1s, coff0=coff0, coff1=coff1, C0=C0, C1=C1,
                 groups=groups)

    per_core = []
    for c in range(n_cores):
        sel, ld, rs, half, order, sortpos, sp = per_core_raw[c]
        t_of = sp // 128
        d_of = sp % 128
        # rank of each edge within its (dst, half) group
        ek = np.lexsort((rs, half, sp))      # group edges by (sp, half)
        key = sp[ek] * 2 + half[ek]
        grp_start = np.concatenate([[0], np.nonzero(np.diff(key))[0] + 1])
        gid = np.zeros(len(ek), np.int64)
        gid[grp_start[1:]] = 1
        gid = np.cumsum(gid)
        rank = np.arange(len(ek)) - grp_start[gid]
        kk = np.empty(len(ek), np.int64)
        kk[ek] = rank

        idx0 = np.zeros(C0 * 128, np.int16)
        idx1 = np.zeros(C1 * 128, np.int16)
        pb0 = np.full((128, C0), PAD_BIAS, np.float32)
        pb1 = np.full((128, C1), PAD_BIAS, np.float32)
        h0 = half == 0
        col0 = coff0[t_of[h0]] + kk[h0]
        idx0[col0 * 128 + d_of[h0]] = rs[h0].astype(np.int16)
        pb0[d_of[h0], col0] = 0.0
        h1 = half == 1
        col1 = coff1[t_of[h1]] + kk[h1]
        idx1[col1 * 128 + d_of[h1]] = (rs[h1] - HALF).astype(np.int16)
        pb1[d_of[h1], col1] = 0.0

        per_core.append(dict(order=order, sortpos=sortpos,
                             idx0=_wrap16(idx0), idx1=_wrap16(idx1),
                             pb0=pb0, pb1=pb1))
    return sched, per_core


def _d_idx(order):
    """[16, NLP/16] wrap of the local row ids to D-gather, -1 trailing pad."""
    flat = np.full(NLP, -1, np.int16)
    flat[:NL] = order.astype(np.int16)
    return _wrap16(flat)


# ---------------------------------------------------------------- bass builder
def build_graph(s0, s1):
    """Build the SPMD bass graph given the two layers' shared schedules."""
    from concourse import bass, bacc, tile
    from concourse import mybir

    _QCTR[0] = 0
    global F32, BF16, I16
    F32 = mybir.dt.float32
    BF16 = mybir.dt.bfloat16
    I16 = mybir.dt.int16
    AF = mybir.ActivationFunctionType
    OP = mybir.AluOpType

    nc = bacc.Bacc("TRN2", target_bir_lowering=False, debug=False,
                   num_devices=N_CORES,
                   dynamic_dma_scratch_size=int(os.environ.get("GNN_SCR", "16384")),
                   num_swdge_queues=int(os.environ.get("GNN_NSWQ", "4")))

    # ---------------- parameters (per-core shards prepared on host)
    xT = nc.declare_dram_parameter("xT", [512, NLP], I16, isOutput=False)  # bf16 bits
    m_wrap = nc.declare_dram_parameter("m_wrap", [128, T], F32, isOutput=False)
    v_wrap = nc.declare_dram_parameter("v_wrap", [128, T], F32, isOutput=False)
    W_o = nc.declare_dram_parameter("W_o", [256, HID], F32, isOutput=False)
    W_u = nc.declare_dram_parameter("W_u", [256, HID], F32, isOutput=False)
    w4 = nc.declare_dram_parameter("w4", [HID, 4], F32, isOutput=False)
    W_diff = nc.declare_dram_parameter("W_diff", [HID, HID], F32, isOutput=False)
    b_diff = nc.declare_dram_parameter("b_diff", [1, HID], F32, isOutput=False)
    W_g = nc.declare_dram_parameter("W_g", [2 * HID, HID], F32, isOutput=False)
    ident = nc.declare_dram_parameter("ident", [128, 128], F32, isOutput=False)
    mask16 = nc.declare_dram_parameter("mask16", [128, 128], I16, isOutput=False)

    idx0_l0 = nc.declare_dram_parameter("idx0_l0", [128, s0["C0"] * 8], I16, False)
    idx1_l0 = nc.declare_dram_parameter("idx1_l0", [128, s0["C1"] * 8], I16, False)
    pb0_l0 = nc.declare_dram_parameter("pb0_l0", [128, s0["C0"]], F32, False)
    pb1_l0 = nc.declare_dram_parameter("pb1_l0", [128, s0["C1"]], F32, False)
    idxd_l0 = nc.declare_dram_parameter("idxd_l0", [128, NLP // 16], I16, False)
    idx0_l1 = nc.declare_dram_parameter("idx0_l1", [128, s1["C0"] * 8], I16, False)
    idx1_l1 = nc.declare_dram_parameter("idx1_l1", [128, s1["C1"] * 8], I16, False)
    pb0_l1 = nc.declare_dram_parameter("pb0_l1", [128, s1["C0"]], F32, False)
    pb1_l1 = nc.declare_dram_parameter("pb1_l1", [128, s1["C1"]], F32, False)
    idxd_l1 = nc.declare_dram_parameter("idxd_l1", [128, NLP // 16], I16, False)
    scat_l1 = nc.declare_dram_parameter("scat_l1", [128, NLP // 16], I16, False)
    oms_l1 = nc.declare_dram_parameter("oms_l1", [128, T], F32, False)

    outp = nc.declare_dram_parameter("out", [NLP, 2 * HID], F32, isOutput=True)

    # ---------------- internal DRAM
    t0loc = nc.dram_tensor("t0loc", [NLP, TBL_COLS], BF16)
    t1loc = nc.dram_tensor("t1loc", [NLP, TBL_COLS], BF16)
    t0fa = nc.dram_tensor("t0fa", [HALF, TBL_COLS], BF16, addr_space="Shared")
    t0fb = nc.dram_tensor("t0fb", [HALF, TBL_COLS], BF16, addr_space="Shared")
    t1fa = nc.dram_tensor("t1fa", [HALF, TBL_COLS], BF16, addr_space="Shared")
    t1fb = nc.dram_tensor("t1fb", [HALF, TBL_COLS], BF16, addr_space="Shared")
    arin = nc.dram_tensor("arin", [2, 132], F32)
    arout = nc.dram_tensor("arout", [2, 132], F32, addr_space="Shared")

    RG = [list(range(N_CORES))]

    from concourse import library_config

    with tile.TileContext(nc) as tc:
        nc.gpsimd.load_library(library_config.mlp)
        with tc.tile_pool(name="persist", bufs=1) as pp:
            # persistent small tensors
            id_s = pp.tile([128, 128], F32)
            nc.sync.dma_start(id_s[:], ident[:])
            w4s = pp.tile([HID, 4], F32)
            nc.sync.dma_start(w4s[:], w4[:])
            wd = pp.tile([HID, HID], F32)
            nc.sync.dma_start(wd[:], W_diff[:])
            bd = pp.tile([1, HID], F32)
            nc.sync.dma_start(bd[:], b_diff[:])
            wgt = pp.tile([128, HID], F32)
            nc.sync.dma_start(wgt[:], W_g[0:128, :])
            wgb = pp.tile([128, HID], F32)
            nc.sync.dma_start(wgb[:], W_g[128:256, :])
            m_s = pp.tile([128, T], F32)
            nc.sync.dma_start(m_s[:], m_wrap[:])
            v_s = pp.tile([128, T], F32)
            nc.sync.dma_start(v_s[:], v_wrap[:])
            ones_col = pp.tile([128, 1], F32)
            nc.vector.memset(ones_col[:], 1.0)
            ones_row = pp.tile([1, 128], F32)
            nc.vector.memset(ones_row[:], 1.0)
            ones11 = pp.tile([1, 1], F32)
            nc.vector.memset(ones11[:], 1.0)
            negm = pp.tile([128, T], F32)
            nc.vector.tensor_scalar_mul(negm[:], m_s[:], -1.0)
            om_s = pp.tile([128, T], F32)     # (1-m)*valid = valid - m
            nc.vector.tensor_sub(om_s[:], v_s[:], m_s[:])
            mm2 = pp.tile([128, T, 2], F32)   # lhsT slices for deltaX partials
            nc.vector.tensor_copy(mm2[:, :, 0], m_s[:])
            nc.vector.tensor_copy(mm2[:, :, 1], om_s[:])

            # prebuilt matmul RHS: RA_c = [W_o chunk | score vec cols], RU, WuG
            with tc.tile_pool(name="dense", bufs=1) as dp, \
                 tc.tile_pool(name="densepsum", bufs=2, space="PSUM") as pmm, \
                 tc.tile_pool(name="psum1", bufs=1, space="PSUM") as p1, \
                 tc.tile_pool(name="xtp", bufs=1) as xp:

                # bf16 weights: load f32, cast once
                RAf = [dp.tile([128, HID], F32, name=f"RAf{c}") for c in range(2)]
                RUf = [dp.tile([128, HID], F32, name=f"RUf{c}") for c in range(2)]
                RA = [dp.tile([128, HID + 4], BF16, name=f"RA{c}") for c in range(2)]
                RU = [dp.tile([128, HID], BF16, name=f"RU{c}") for c in range(2)]
                WuG = [dp.tile([128, HID], BF16, name=f"WuG{c}") for c in range(2)]
                wgt16 = dp.tile([128, HID], BF16, tag="wgt16")
                nc.vector.tensor_copy(wgt16[:], wgt[:])
                w4s16 = dp.tile([HID, 4], BF16, tag="w4s16")
                nc.vector.tensor_copy(w4s16[:], w4s[:])
                for c in range(2):
                    nc.sync.dma_start(RAf[c][:], W_o[128 * c:128 * (c + 1), :])
                    nc.sync.dma_start(RUf[c][:], W_u[128 * c:128 * (c + 1), :])
                    nc.vector.tensor_copy(RA[c][:, 0:HID], RAf[c][:])
                    nc.vector.tensor_copy(RU[c][:], RUf[c][:])
                # wv = W_o @ w4 (via W_o^T chunks), WuG = W_u @ Wg_top
                for c in range(2):
                    ptp = pmm.tile([128, 128], F32, tag="a")
                    nc.tensor.transpose(ptp[:], RAf[c][:], id_s[:])
                    wt = dp.tile([128, 128], BF16, tag="wt")
                    nc.vector.tensor_copy(wt[:], ptp[:])
                    pwv = pmm.tile([128, 4], F32, tag="b")
                    nc.tensor.matmul(pwv[:], wt[:], w4s16[:], start=True, stop=True)
                    nc.vector.tensor_copy(RA[c][:, HID:HID + 4], pwv[:])

                    ptp2 = pmm.tile([128, 128], F32, tag="a")
                    nc.tensor.transpose(ptp2[:], RUf[c][:], id_s[:])
                    wt2 = dp.tile([128, 128], BF16, tag="wt")
                    nc.vector.tensor_copy(wt2[:], ptp2[:])
                    pwg = pmm.tile([128, 128], F32, tag="b")
                    nc.tensor.matmul(pwg[:], wt2[:], wgt16[:], start=True, stop=True)
                    nc.vector.tensor_copy(WuG[c][:], pwg[:])

                # resident transposed x shard, bf16: 4 chunks of [128, NLP]
                xTc = [xp.tile([128, NLP], BF16, name=f"xTc{c}") for c in range(4)]
                for c in range(4):
                    nc.sync.dma_start(xTc[c][:], xT[128 * c:128 * (c + 1), :].bitcast(BF16))

                xo_s = dp.tile([128, T, HID + 4], F32, tag="xo_s")
                xu_s = dp.tile([128, T, HID], F32, tag="xu_s")
                pdx = p1.tile([2, 128], F32, tag="pdx")
                pcnt = p1.tile([2, 1], F32, tag="pcnt")

                # ---- dense pass A
                for t in range(T):
                    sl = slice(128 * t, 128 * (t + 1))
                    po = pmm.tile([128, HID + 4], F32, tag="a")
                    nc.tensor.matmul(po[:], xTc[0][:, sl], RA[0][:],
                                     start=True, stop=False)
                    nc.tensor.matmul(po[:], xTc[1][:, sl], RA[1][:],
                                     start=False, stop=True)
                    pu = pmm.tile([128, HID], F32, tag="b")
                    nc.tensor.matmul(pu[:], xTc[2][:, sl], RU[0][:],
                                     start=True, stop=False)
                    nc.tensor.matmul(pu[:], xTc[3][:, sl], RU[1][:],
                                     start=False, stop=True)
                    nc.scalar.copy(xo_s[:, t, :], po[:])
                    nc.vector.tensor_copy(xu_s[:, t, :], pu[:])
                    t1t = dp.tile([128, HID], F32, tag="t1t", bufs=3)
                    nc.vector.tensor_scalar_mul(t1t[:], xu_s[:, t, :],
                                                m_s[:, t:t + 1])
                    # deltaX partials: [m, valid-m]^T @ [x_o], and counts
                    nc.tensor.matmul(pdx[:], mm2[:, t, :], xo_s[:, t, 0:HID],
                                     start=(t == 0), stop=(t == T - 1))
                    nc.tensor.matmul(pcnt[:], mm2[:, t, :], ones_col[:],
                                     start=(t == 0), stop=(t == T - 1))
                    # output cols 0:128 = x_o ; cols 128:256 = x_u*m (+= xuhat later)
                    nc.sync.dma_start(outp[sl, 0:HID], xo_s[:, t, 0:HID])
                    nc.sync.dma_start(outp[sl, HID:2 * HID], t1t[:])

                # ---- deltaX AllReduce
                ar_s = dp.tile([2, 132], F32, tag="ar_s")
                nc.vector.memset(ar_s[:], 0.0)
                nc.vector.tensor_copy(ar_s[:, 0:128], pdx[:])
                nc.vector.tensor_copy(ar_s[:, 128:129], pcnt[:])
                nc.sync.dma_start(arin[:], ar_s[:])
                nc.gpsimd.collective_compute(
                    "AllReduce", OP.add, ins=[arin[:]], outs=[arout[:]],
                    replica_groups=RG)
                ars = dp.tile([2, 132], F32, tag="ars")
                nc.sync.dma_start(ars[:], arout[:])
                rec = dp.tile([2, 1], F32, tag="rec")
                nc.vector.reciprocal(rec[:], ars[:, 128:129])
                means = dp.tile([2, 128], F32, tag="means")
                nc.vector.tensor_scalar_mul(means[:], ars[:, 0:128], rec[:])
                ptp3 = pmm.tile([128, 128], F32, tag="a")
                nc.tensor.transpose(ptp3[:, 0:2], means[:], id_s[0:2, 0:2])
                mT = dp.tile([128, 2], F32, tag="mT")
                nc.vector.tensor_copy(mT[:], ptp3[:, 0:2])
                dxc = dp.tile([128, 1], F32, tag="dxc")
                nc.vector.tensor_sub(dxc[:], mT[:, 0:1], mT[:, 1:2])
                pad_ = pmm.tile([1, 128], F32, tag="b")
                nc.tensor.matmul(pad_[:], dxc[:], wd[:], start=True, stop=True)
                adr = dp.tile([1, 128], F32, tag="adr")
                nc.vector.tensor_add(adr[:], pad_[:], bd[:])
                pac = pmm.tile([128, 1], F32, tag="a")
                nc.tensor.matmul(pac[:], adr[:], ones11[:], start=True, stop=True)
                adc = dp.tile([128, 1], F32, tag="adc")
                nc.vector.tensor_copy(adc[:], pac[:])
                pcr = pmm.tile([1, 128], F32, tag="b")
                nc.tensor.matmul(pcr[:], adc[:], wgb[:], start=True, stop=True)
                crow = dp.tile([1, 128], F32, tag="crow")
                nc.vector.tensor_copy(crow[:], pcr[:])

                # ---- dense pass B: message_u + scores -> full t0loc rows
                t0st = dp.tile([128, T, TBL_COLS], BF16, tag="t0st")
                nc.vector.memset(t0st[:], 0.0)
                nc.vector.tensor_copy(
                    t0st[:, :, 128:136].bitcast(F32),
                    xo_s[:, :, HID:HID + 4])
                for t in range(T):
                    sl = slice(128 * t, 128 * (t + 1))
                    pD = pmm.tile([128, HID], F32, tag="a")
                    nc.tensor.matmul(pD[:], xTc[2][:, sl], WuG[0][:],
                                     start=True, stop=False)
                    nc.tensor.matmul(pD[:], xTc[3][:, sl], WuG[1][:],
                                     start=False, stop=False)
                    nc.tensor.matmul(pD[:], ones_row[:], crow[:],
                                     start=False, stop=True)
                    nc.vector.scalar_tensor_tensor(
                        t0st[:, t, 0:HID], pD[:], negm[:, t:t + 1], xu_s[:, t, :],
                        op0=OP.mult, op1=OP.add)
                    # 4 f32 scores bit-stored in bf16 cols 128:136


                t0dst = bass.AP(t0loc, 0, [[TBL_COLS, 128], [128 * TBL_COLS, T],
                                           [1, TBL_COLS]])
                nc.sync.dma_start(t0dst, t0st[:])

            # ---------------- AllGather table0; edge layers
            phase = os.environ.get("GNN_PHASE", "full")
            if phase != "dense":
                nc.gpsimd.collective_compute(
                    "AllGather", OP.bypass, ins=[t0loc[0:HROWS, :]],
                    outs=[t0fa[:]], replica_groups=RG)
                nc.gpsimd.collective_compute(
                    "AllGather", OP.bypass, ins=[t0loc[HROWS:NLP, :]],
                    outs=[t0fb[:]], replica_groups=RG)

            if phase not in ("dense", "ag"):
                _edge_layer(nc, tc, 0, s0, (t0fa, t0fb), t0loc, t1loc,
                            idx0_l0, idx1_l0, pb0_l0, pb1_l0, idxd_l0,
                            None, None, mask16, outp)

            if phase == "full":
                nc.gpsimd.collective_compute(
                    "AllGather", OP.bypass, ins=[t1loc[0:HROWS, :]],
                    outs=[t1fa[:]], replica_groups=RG)
                nc.gpsimd.collective_compute(
                    "AllGather", OP.bypass, ins=[t1loc[HROWS:NLP, :]],
                    outs=[t1fb[:]], replica_groups=RG)

                _edge_layer(nc, tc, 1, s1, (t1fa, t1fb), t1loc, None,
                            idx0_l1, idx1_l1, pb0_l1, pb1_l1, idxd_l1,
                            scat_l1, oms_l1, mask16, outp)

    nc.finalize()
    return nc


def _edge_layer(nc, tc, layer, sched, tfull, tloc, tnext,
                idx0p, idx1p, pb0p, pb1p, idxdp, scatp, omsp, mask16p, outp):
    from concourse import bass, mybir
    OP = mybir.AluOpType
    AF = mybir.ActivationFunctionType
    F32 = mybir.dt.float32
    BF16 = mybir.dt.bfloat16
    I16 = mybir.dt.int16

    K0, K1 = sched["K0"], sched["K1"]
    coff0, coff1 = sched["coff0"], sched["coff1"]
    C0, C1 = sched["C0"], sched["C1"]
    groups = sched["groups"]
    g0max = max(int(sum(K0[t] for t in g)) for g in groups)
    g1max = max(int(sum(K1[t] for t in g)) for g in groups)

    with tc.tile_pool(name=f"edge{layer}", bufs=1) as ep, \
         tc.tile_pool(name=f"gath{layer}", bufs=2) as gp, \
         tc.tile_pool(name=f"work{layer}", bufs=3) as wp, \
         tc.tile_pool(name=f"epsum{layer}", bufs=1, space="PSUM") as pp2:

        idx0 = ep.tile([128, C0 * 8], I16)
        nc.sync.dma_start(idx0[:], idx0p[:])
        idx1 = ep.tile([128, C1 * 8], I16)
        nc.sync.dma_start(idx1[:], idx1p[:])
        pb0 = ep.tile([128, C0], F32)
        nc.sync.dma_start(pb0[:], pb0p[:])
        pb1 = ep.tile([128, C1], F32)
        nc.sync.dma_start(pb1[:], pb1p[:])
        idxd = ep.tile([128, NLP // 16], I16)
        nc.sync.dma_start(idxd[:], idxdp[:])
        mask = ep.tile([128, 128], BF16)
        nc.sync.dma_start(mask[:], mask16p[:].bitcast(BF16))
        if layer == 1:
            scat = ep.tile([128, NLP // 16], I16)
            nc.sync.dma_start(scat[:], scatp[:])
            oms = ep.tile([128, T], F32)
            nc.sync.dma_start(oms[:], omsp[:])

        # gather all dst rows once: a_tar (+ next-layer scores on layer 0)
        # (dma_gather calls are capped at 1024 indices: larger calls overflow
        #  the Q7-local index scratch and hard-crash the device)
        D = ep.tile([128, T, TBL_COLS], BF16)
        for c0 in range(0, T, 8):
            c1 = min(c0 + 8, T)
            ni = (c1 - c0) * 128
            nv = max(0, min(NL - c0 * 128, ni))
            nc.gpsimd.dma_gather(D[:, c0:c1, :], tloc[:],
                                 idxd[:, c0 * 8:c1 * 8], ni, nv, TBL_COLS,
                                 queue_num=_nq())

        if layer == 0:
            msgst = ep.tile([128, T, TBL_COLS], BF16)
            nc.vector.memset(msgst[:], 0.0)
            # next-layer scores ride along in the D rows: one batched copy
            nc.vector.tensor_copy(msgst[:, :, 128:132], D[:, :, 132:136])

        for g in groups:
            gc00, gc01 = int(coff0[g[0]]), int(coff0[g[-1] + 1])
            gc10, gc11 = int(coff1[g[0]]), int(coff1[g[-1] + 1])
            n0, n1 = gc01 - gc00, gc11 - gc10
            G0 = gp.tile([128, g0max, TBL_COLS], BF16, tag="G0")
            G1 = gp.tile([128, g1max, TBL_COLS], BF16, tag="G1")
            tfa, tfb = tfull
            for cc0 in range(0, n0, 8):
                cc1 = min(cc0 + 8, n0)
                nc.gpsimd.dma_gather(
                    G0[:, cc0:cc1, :], tfa[:],
                    idx0[:, (gc00 + cc0) * 8:(gc00 + cc1) * 8],
                    (cc1 - cc0) * 128, (cc1 - cc0) * 128, TBL_COLS,
                    queue_num=_nq())
            for cc0 in range(0, n1, 8):
                cc1 = min(cc0 + 8, n1)
                nc.gpsimd.dma_gather(
                    G1[:, cc0:cc1, :], tfb[:],
                    idx1[:, (gc10 + cc0) * 8:(gc10 + cc1) * 8],
                    (cc1 - cc0) * 128, (cc1 - cc0) * 128, TBL_COLS,
                    queue_num=_nq())

            if layer == 1:
                scst = gp.tile([128, len(g), HID], F32, tag="scst")

            for ti, t in enumerate(g):
                k0, k1 = int(K0[t]), int(K1[t])
                o0, o1 = int(coff0[t]) - gc00, int(coff1[t]) - gc10
                atar = D[:, t, 130:132].bitcast(F32)
                z0 = wp.tile([128, 1], F32, tag="z0")
                z1 = wp.tile([128, 1], F32, tag="z1")
                ps = []
                for (h, kh, oh, G, pb, co) in (
                        (0, k0, o0, G0, pb0, int(coff0[t])),
                        (1, k1, o1, G1, pb1, int(coff1[t]))):
                    zh = (z0, z1)[h]
                    if kh == 0:
                        nc.vector.memset(zh[:], 0.0)
                        ps.append(None)
                        continue
                    sb = wp.tile([128, kh], F32, tag=f"sb{h}")
                    nc.vector.scalar_tensor_tensor(
                        sb[:], G[:, oh:oh + kh, 128:130].bitcast(F32)[:, :, 0],
                        atar, pb[:, co:co + kh], op0=OP.add, op1=OP.add)
                    nc.vector.scalar_tensor_tensor(
                        sb[:], sb[:], NEG_SLOPE, sb[:], op0=OP.mult, op1=OP.max)
                    ph = wp.tile([128, kh], F32, tag=f"p{h}")
                    nc.scalar.activation(ph[:], sb[:], AF.Exp, accum_out=zh[:])
                    ps.append(ph)
                z = wp.tile([128, 1], F32, tag="z")
                nc.vector.scalar_tensor_tensor(
                    z[:], z0[:], Z_EPS, z1[:], op0=OP.add, op1=OP.add)
                rz = wp.tile([128, 1], F32, tag="rz")
                nc.vector.reciprocal(rz[:], z[:])
                if layer == 1:
                    nc.vector.tensor_mul(rz[:], rz[:], oms[:, t:t + 1])
                # fold 1/Z (and (1-m) on layer 1) into the edge weights
                if ps[0] is not None:
                    nc.vector.tensor_scalar_mul(ps[0][:], ps[0][:], rz[:])
                if ps[1] is not None:
                    nc.vector.tensor_scalar_mul(ps[1][:], ps[1][:], rz[:])

                if layer == 0:
                    dest = msgst[:, t, 0:HID]
                else:
                    dest = scst[:, ti, :]
                chunks = ([(0, k) for k in range(k0)] +
                          [(1, k) for k in range(k1)])
                if not chunks:
                    if layer == 1:
                        nc.vector.memset(dest, 0.0)
                    continue
                pacc = pp2.tile([128, HID], F32, tag="pacc", bufs=2)
                for j, (h, k) in enumerate(chunks):
                    G, oh = (G0, o0) if h == 0 else (G1, o1)
                    ph = ps[h]
                    db = wp.tile([128, 128], BF16, tag="db", bufs=4)
                    nc.scalar.activation(db[:], mask[:], AF.Copy,
                                         scale=ph[:, k:k + 1])
                    nc.tensor.matmul(pacc[:], db[:], G[:, oh + k, 0:HID],
                                     start=(j == 0),
                                     stop=(j == len(chunks) - 1))
                nc.vector.tensor_copy(dest, pacc[:])

            if layer == 1:
                lo = g[0] * 128
                hi = min((g[-1] + 1) * 128, NL)
                nc.gpsimd.dma_scatter_add(
                    outp[:, HID:2 * HID], scst[:, 0:len(g), :],
                    scat[:, lo // 16:(g[-1] + 1) * 128 // 16],
                    len(g) * 128, hi - lo, HID, elem_step=2 * HID,
                    queue_num=_nq())

        if layer == 0:
            tA = HROWS // 128            # 24.5 -> 24 full tiles in half A
            d1 = bass.AP(tnext, 0,
                         [[TBL_COLS, 128], [128 * TBL_COLS, tA], [1, TBL_COLS]])
            nc.sync.dma_start(d1, msgst[:, 0:tA, :])
            d2 = bass.AP(tnext, tA * 128 * TBL_COLS,
                         [[TBL_COLS, 128], [128 * TBL_COLS, 1], [1, TBL_COLS]])
            nc.sync.dma_start(d2, msgst[:, tA:tA + 1, :])
            d3 = bass.AP(tnext, (tA + 1) * 128 * TBL_COLS,
                         [[TBL_COLS, 128], [128 * TBL_COLS, T - tA - 1],
                          [1, TBL_COLS]])
            nc.sync.dma_start(d3, msgst[:, tA + 1:T, :])


# ---------------------------------------------------------------- host driver
def _prep_inputs(x, central_mask, edge_index0, edge_index1,
                 W_o, W_u, w_src0, w_tar0, w_src1, w_tar1, W_diff, b_diff, W_g):
    """Shard + route everything; returns (s0, s1, in_maps)."""
    x = np.asarray(x, np.float32)
    m = np.asarray(central_mask, np.int32)
    e0 = np.asarray(edge_index0, np.int64)
    e1 = np.asarray(edge_index1, np.int64)

    # layer-0 table rows are node-ordered: r0(v) = (v//NL)*NLP + v%NL
    def r0(v):
        return (v // NL) * NLP + (v % NL)

    s0, pc0 = _route_layer(r0(e0[0]), e0[1])

    # layer-1 table rows are in each owner core's layer-0 sorted order
    sp0 = np.concatenate([pc0[c]["sortpos"] for c in range(N_CORES)])

    def r1(v):
        return (v // NL) * NLP + sp0[v]

    s1, pc1 = _route_layer(r1(e1[0]), e1[1])

    w4 = np.stack([np.asarray(w, np.float32) for w in
                   (w_src0, w_tar0, w_src1, w_tar1)], axis=1)
    ident = np.eye(128, dtype=np.float32)

    in_maps = []
    for c in range(N_CORES):
        xs = x[NL * c:NL * (c + 1)]
        xp = np.zeros((NLP, 512), np.float32)
        xp[:NL] = xs
        ms = np.zeros(NLP, np.float32)
        ms[:NL] = m[NL * c:NL * (c + 1)]
        vs = np.zeros(NLP, np.float32)
        vs[:NL] = 1.0
        o1 = pc1[c]["order"]
        omsv = np.zeros(NLP, np.float32)
        omsv[:NL] = 1.0 - ms[o1]
        scat_flat = np.full(NLP, -1, np.int16)
        scat_flat[:NL] = o1.astype(np.int16)
        # layer-1 D-gather reads t1loc rows = layer-0 sorted positions
        idxd1_flat = np.full(NLP, -1, np.int16)
        idxd1_flat[:NL] = pc0[c]["sortpos"][o1].astype(np.int16)

        in_maps.append({
            "xT": np.ascontiguousarray(xp.T).astype(
                ml_dtypes.bfloat16).view(np.int16),
            "m_wrap": np.ascontiguousarray(ms.reshape(T, 128).T),
            "v_wrap": np.ascontiguousarray(vs.reshape(T, 128).T),
            "W_o": np.asarray(W_o, np.float32),
            "W_u": np.asarray(W_u, np.float32),
            "w4": w4,
            "W_diff": np.asarray(W_diff, np.float32),
            "b_diff": np.asarray(b_diff, np.float32).reshape(1, HID),
            "W_g": np.asarray(W_g, np.float32),
            "ident": ident,
            "mask16": np.eye(128, dtype=ml_dtypes.bfloat16).view(np.int16),
            "idx0_l0": pc0[c]["idx0"], "idx1_l0": pc0[c]["idx1"],
            "pb0_l0": pc0[c]["pb0"], "pb1_l0": pc0[c]["pb1"],
            "idxd_l0": _d_idx(pc0[c]["order"]),
            "idx0_l1": pc1[c]["idx0"], "idx1_l1": pc1[c]["idx1"],
            "pb0_l1": pc1[c]["pb0"], "pb1_l1": pc1[c]["pb1"],
            "idxd_l1": _wrap16(idxd1_flat),
            "scat_l1": _wrap16(scat_flat),
            "oms_l1": np.ascontiguousarray(omsv.reshape(T, 128).T),
        })
    return s0, s1, in_maps


_CACHE = {}


def _install_ntff_hook():
    """Register the axon NTFF profiling hook if the image's antenv lacks it."""
    import types
    import contextlib
    try:
        from antenv.axon_hooks import get_axon_ntff_profile_hook  # noqa: F401
        return True
    except ImportError:
        pass
    try:
        if "/root/.axon_site" not in sys.path:
            sys.path.append("/root/.axon_site")
        from trn_agent_boot.trn_boot import _ntff_profile_via_ctypes
        import antenv
        hook = _ntff_profile_via_ctypes("/opt/axon/libaxon_pjrt.so")
        mod = types.ModuleType("antenv.axon_hooks")
        _h = [hook]
        mod.set_axon_ntff_profile_hook = lambda h: _h.__setitem__(0, h)
        mod.get_axon_ntff_profile_hook = lambda: _h[0]
        sys.modules["antenv.axon_hooks"] = mod
        antenv.axon_hooks = mod
        # artifact upload has no bucket in this container; stub it out
        from concourse import bass_utils as _bu
        _bu.upload_artifacts = lambda tmpdir: "local"
        return hook is not None
    except Exception as e:
        print("ntff hook install failed:", e)
        return False


def kernel(**inputs):
    s0, s1, in_maps = _prep_inputs(**inputs)

    from concourse.bass_utils import run_bass_kernel_spmd

    key = (tuple(s0["K0"]), tuple(s0["K1"]), tuple(s1["K0"]), tuple(s1["K1"]))
    if key not in _CACHE:
        _CACHE[key] = build_graph(s0, s1)
    nc = _CACHE[key]

    trace = bool(int(os.environ.get("GNN_TRACE", "0")))
    if trace:
        trace = _install_ntff_hook()
    res = run_bass_kernel_spmd(nc, in_maps, list(range(N_CORES)), trace=trace)
    if trace and res.exec_time_ns is not None:
        print(f"HW exec time: {res.exec_time_ns} ns")
        kernel.last_exec_ns = res.exec_time_ns
        kernel.last_profile = res.profile_json
    out = np.concatenate([res.results[c]["out"][:NL] for c in range(N_CORES)], 0)
    return out.astype(np.float32)


if __name__ == "__main__":
    import reference
    inp = {k: np.asarray(v) for k, v in reference.setup_inputs().items()}
    exp = np.asarray(reference.reference(**inp))
    act = kernel(**inp)
    err = np.abs(act - exp)
    rel = np.linalg.norm(act - exp) / np.linalg.norm(exp)
    print("max abs err:", err.max(), "rel:", rel)



# revision 22
# speedup vs baseline: 1.5021x; 1.1688x over previous
"""Trainium2 8-core Bass kernel for the Adapted_complementor GNN (2-layer GAT).

Strategy (graph/data parallel per sharding hint):
  - nodes sharded contiguously: core c owns [NL*c, NL*(c+1)), padded to NLP rows
  - dense phase per core: x_o/x_u/scores/message_u via TensorE matmuls on a
    host-pre-transposed x shard; deltaX via tiny AllReduce
  - per layer: bf16 "table" row per node [msg(128bf16)|4 scores f32|pad] = 512B,
    AllGather -> every core holds the full table; edges routed by dst on host,
    one 512B dma_gather per edge slot fetches msg+score together
  - segment softmax without max-subtraction (value range is tiny; exact in f32)
  - per-dst-tile [128 dst x K] layout; a_tar is a per-partition scalar; pads get
    a -1e4 additive bias (exp -> 0); 1/Z folded into the final per-tile scale
  - int16 gather-index limit: table split into two 25088-row halves, per-half
    slot streams per tile
  - layer-1 output blended with (1-m) and scatter-added into the out rows
"""

import os
import sys
import numpy as np

for p in ("/opt/trn_rl_repo", "/root/.axon_site/_ro/trn_rl_repo"):
    if os.path.isdir(p) and p not in sys.path:
        sys.path.insert(0, p)

import ml_dtypes  # noqa: E402  (ships with jax)

# ---------------------------------------------------------------- constants
N_CORES = 8
N_NODES = int(os.environ.get("GNN_N", "50000"))
N_EDGES = int(os.environ.get("GNN_E", "800000"))
DIM_O = 256
DIM_U = 256
HID = 128
NEG_SLOPE = 0.1

NL = N_NODES // N_CORES            # 6250 real nodes per core
T = (NL + 127) // 128              # 49 dst tiles per core
NLP = T * 128                      # 6272 padded rows per core
NG = NLP * N_CORES                 # 50176 global table rows
HROWS = NLP // 2                   # 3136 local rows per half-table
HALF = HROWS * N_CORES             # 25088 rows per half (< 32768: int16-safe)
TBL_COLS = 256                     # bytes per table row: 128 fp8e3 msg + 4 f32 scores + pad
PAD_BIAS = -1.0e4
Z_EPS = 1e-30
GROUP_COL_BUDGET = int(os.environ.get("GNN_GCB", "96"))             # max summed K-cols (both halves) per gather group

F32 = None  # filled after bass import
BF16 = None
I16 = None
U8 = None
FP8 = None

NSWQ = int(os.environ.get("GNN_NSWQ", "4"))
_QCTR = [0]


def _nq():
    """Round-robin SWDGE queue id for gather/scatter calls."""
    q = _QCTR[0] % NSWQ
    _QCTR[0] += 1
    return q


def _wrap16(flat):
    """Pack a flat int16 index list into the [128, n/16] SBUF wrap layout
    (16-partition wrap, replicated 8x across the partition groups)."""
    flat = np.asarray(flat, dtype=np.int16)
    n = len(flat)
    assert n % 16 == 0, n
    w = np.ascontiguousarray(flat.reshape(n // 16, 16).T)
    return np.tile(w, (8, 1))


# ---------------------------------------------------------------- host routing
def _route_layer(src_rows, dst, n_cores=N_CORES):
    """Route one layer's edges.

    src_rows: global table-row id of each edge's source (w.r.t. the table the
              layer gathers from). dst: global node id of each edge's dest.
    Returns (sched, per_core) where sched has the SPMD-shared tile schedule and
    per_core the per-core index/bias arrays.
    """
    dst_core = dst // NL
    dst_loc = dst % NL
    # src_rows: (core, local) pairs encoded as core*NLP+local; re-encode into
    # half-tables split by local row (< HROWS -> A, else B)
    s_core = src_rows // NLP
    s_loc = src_rows % NLP
    in_b = s_loc >= HROWS
    src_rows = np.where(in_b, s_core * HROWS + (s_loc - HROWS) + HALF,
                        s_core * HROWS + s_loc)

    per_core_raw = []
    K0 = np.zeros((n_cores, T), np.int64)
    K1 = np.zeros((n_cores, T), np.int64)
    for c in range(n_cores):
        sel = np.nonzero(dst_core == c)[0]
        ld = dst_loc[sel]
        rs = src_rows[sel]
        half = (rs >= HALF).astype(np.int64)
        c0 = np.bincount(ld[half == 0], minlength=NL)
        c1 = np.bincount(ld[half == 1], minlength=NL)
        # descending lexsort by c0, snake on c1 within equal-c0 runs:
        # keeps both per-tile maxima tight
        order = np.lexsort(((-1) ** (c0 % 2) * -c1, -c0))
        sortpos = np.empty(NL, np.int64)
        sortpos[order] = np.arange(NL)
        sp = sortpos[ld]
        for t in range(T):
            lo, hi = t * 128, min((t + 1) * 128, NL)
            K0[c, t] = c0[order[lo:hi]].max(initial=0)
            K1[c, t] = c1[order[lo:hi]].max(initial=0)
        per_core_raw.append((sel, ld, rs, half, order, sortpos, sp))

    K0s = K0.max(axis=0)
    K1s = K1.max(axis=0)
    coff0 = np.concatenate([[0], np.cumsum(K0s)])
    coff1 = np.concatenate([[0], np.cumsum(K1s)])
    C0, C1 = int(coff0[-1]), int(coff1[-1])

    # greedy tile grouping under the SBUF column budget
    groups = []
    cur, cost = [], 0
    for t in range(T):
        w = int(K0s[t] + K1s[t])
        if cur and (cost + w > GROUP_COL_BUDGET or len(cur) >= 8):
            groups.append(cur)
            cur, cost = [], 0
        cur.append(t)
        cost += w
    if cur:
        groups.append(cur)

    sched = dict(K0=K0s, K1=K1s, coff0=coff0, coff1=coff1, C0=C0, C1=C1,
                 groups=groups)

    per_core = []
    for c in range(n_cores):
        sel, ld, rs, half, order, sortpos, sp = per_core_raw[c]
        t_of = sp // 128
        d_of = sp % 128
        # rank of each edge within its (dst, half) group
        ek = np.lexsort((rs, half, sp))      # group edges by (sp, half)
        key = sp[ek] * 2 + half[ek]
        grp_start = np.concatenate([[0], np.nonzero(np.diff(key))[0] + 1])
        gid = np.zeros(len(ek), np.int64)
        gid[grp_start[1:]] = 1
        gid = np.cumsum(gid)
        rank = np.arange(len(ek)) - grp_start[gid]
        kk = np.empty(len(ek), np.int64)
        kk[ek] = rank

        idx0 = np.zeros(C0 * 128, np.int16)
        idx1 = np.zeros(C1 * 128, np.int16)
        pb0 = np.full((128, C0), PAD_BIAS, np.float32)
        pb1 = np.full((128, C1), PAD_BIAS, np.float32)
        h0 = half == 0
        col0 = coff0[t_of[h0]] + kk[h0]
        idx0[col0 * 128 + d_of[h0]] = rs[h0].astype(np.int16)
        pb0[d_of[h0], col0] = 0.0
        h1 = half == 1
        col1 = coff1[t_of[h1]] + kk[h1]
        idx1[col1 * 128 + d_of[h1]] = (rs[h1] - HALF).astype(np.int16)
        pb1[d_of[h1], col1] = 0.0

        per_core.append(dict(order=order, sortpos=sortpos,
                             idx0=_wrap16(idx0), idx1=_wrap16(idx1),
                             pb0=pb0, pb1=pb1))
    return sched, per_core


def _d_idx(order):
    """[16, NLP/16] wrap of the local row ids to D-gather, -1 trailing pad."""
    flat = np.full(NLP, -1, np.int16)
    flat[:NL] = order.astype(np.int16)
    return _wrap16(flat)


# ---------------------------------------------------------------- bass builder
def build_graph(s0, s1):
    """Build the SPMD bass graph given the two layers' shared schedules."""
    from concourse import bass, bacc, tile
    from concourse import mybir

    _QCTR[0] = 0
    global F32, BF16, I16, U8, FP8
    F32 = mybir.dt.float32
    BF16 = mybir.dt.bfloat16
    I16 = mybir.dt.int16
    U8 = mybir.dt.uint8
    FP8 = mybir.dt.float8e3
    AF = mybir.ActivationFunctionType
    OP = mybir.AluOpType

    nc = bacc.Bacc("TRN2", target_bir_lowering=False, debug=False,
                   num_devices=N_CORES,
                   dynamic_dma_scratch_size=int(os.environ.get("GNN_SCR", "16384")),
                   num_swdge_queues=int(os.environ.get("GNN_NSWQ", "4")))

    # ---------------- parameters (per-core shards prepared on host)
    xT = nc.declare_dram_parameter("xT", [512, NLP], I16, isOutput=False)  # bf16 bits
    m_wrap = nc.declare_dram_parameter("m_wrap", [128, T], F32, isOutput=False)
    v_wrap = nc.declare_dram_parameter("v_wrap", [128, T], F32, isOutput=False)
    W_o = nc.declare_dram_parameter("W_o", [256, HID], F32, isOutput=False)
    W_u = nc.declare_dram_parameter("W_u", [256, HID], F32, isOutput=False)
    w4 = nc.declare_dram_parameter("w4", [HID, 4], F32, isOutput=False)
    W_diff = nc.declare_dram_parameter("W_diff", [HID, HID], F32, isOutput=False)
    b_diff = nc.declare_dram_parameter("b_diff", [1, HID], F32, isOutput=False)
    W_g = nc.declare_dram_parameter("W_g", [2 * HID, HID], F32, isOutput=False)
    ident = nc.declare_dram_parameter("ident", [128, 128], F32, isOutput=False)

    idx0_l0 = nc.declare_dram_parameter("idx0_l0", [128, s0["C0"] * 8], I16, False)
    idx1_l0 = nc.declare_dram_parameter("idx1_l0", [128, s0["C1"] * 8], I16, False)
    pb0_l0 = nc.declare_dram_parameter("pb0_l0", [128, s0["C0"]], F32, False)
    pb1_l0 = nc.declare_dram_parameter("pb1_l0", [128, s0["C1"]], F32, False)
    idxd_l0 = nc.declare_dram_parameter("idxd_l0", [128, NLP // 16], I16, False)
    idx0_l1 = nc.declare_dram_parameter("idx0_l1", [128, s1["C0"] * 8], I16, False)
    idx1_l1 = nc.declare_dram_parameter("idx1_l1", [128, s1["C1"] * 8], I16, False)
    pb0_l1 = nc.declare_dram_parameter("pb0_l1", [128, s1["C0"]], F32, False)
    pb1_l1 = nc.declare_dram_parameter("pb1_l1", [128, s1["C1"]], F32, False)
    idxd_l1 = nc.declare_dram_parameter("idxd_l1", [128, NLP // 16], I16, False)
    scat_l1 = nc.declare_dram_parameter("scat_l1", [128, NLP // 16], I16, False)
    oms_l1 = nc.declare_dram_parameter("oms_l1", [128, T], F32, False)

    outp = nc.declare_dram_parameter("out", [NLP, 2 * HID], F32, isOutput=True)

    # ---------------- internal DRAM (tables are raw bytes: fp8 msg + f32 scores)
    t0loc = nc.dram_tensor("t0loc", [NLP, TBL_COLS], U8)
    t1loc = nc.dram_tensor("t1loc", [NLP, TBL_COLS], U8)
    t0fa = nc.dram_tensor("t0fa", [HALF, TBL_COLS], U8, addr_space="Shared")
    t0fb = nc.dram_tensor("t0fb", [HALF, TBL_COLS], U8, addr_space="Shared")
    t1fa = nc.dram_tensor("t1fa", [HALF, TBL_COLS], U8, addr_space="Shared")
    t1fb = nc.dram_tensor("t1fb", [HALF, TBL_COLS], U8, addr_space="Shared")
    arin = nc.dram_tensor("arin", [2, 132], F32)
    arout = nc.dram_tensor("arout", [2, 132], F32, addr_space="Shared")

    RG = [list(range(N_CORES))]

    from concourse import library_config

    with tile.TileContext(nc) as tc:
        nc.gpsimd.load_library(library_config.mlp)
        with tc.tile_pool(name="persist", bufs=1) as pp:
            # persistent small tensors
            id_s = pp.tile([128, 128], F32)
            nc.sync.dma_start(id_s[:], ident[:])
            w4s = pp.tile([HID, 4], F32)
            nc.sync.dma_start(w4s[:], w4[:])
            wd = pp.tile([HID, HID], F32)
            nc.sync.dma_start(wd[:], W_diff[:])
            bd = pp.tile([1, HID], F32)
            nc.sync.dma_start(bd[:], b_diff[:])
            wgt = pp.tile([128, HID], F32)
            nc.sync.dma_start(wgt[:], W_g[0:128, :])
            wgb = pp.tile([128, HID], F32)
            nc.sync.dma_start(wgb[:], W_g[128:256, :])
            m_s = pp.tile([128, T], F32)
            nc.sync.dma_start(m_s[:], m_wrap[:])
            v_s = pp.tile([128, T], F32)
            nc.sync.dma_start(v_s[:], v_wrap[:])
            ones_col = pp.tile([128, 1], F32)
            nc.vector.memset(ones_col[:], 1.0)
            ones_row = pp.tile([1, 128], F32)
            nc.vector.memset(ones_row[:], 1.0)
            ones11 = pp.tile([1, 1], F32)
            nc.vector.memset(ones11[:], 1.0)
            negm = pp.tile([128, T], F32)
            nc.vector.tensor_scalar_mul(negm[:], m_s[:], -1.0)
            om_s = pp.tile([128, T], F32)     # (1-m)*valid = valid - m
            nc.vector.tensor_sub(om_s[:], v_s[:], m_s[:])
            mm2 = pp.tile([128, T, 2], F32)   # lhsT slices for deltaX partials
            nc.vector.tensor_copy(mm2[:, :, 0], m_s[:])
            nc.vector.tensor_copy(mm2[:, :, 1], om_s[:])

            # prebuilt matmul RHS: RA_c = [W_o chunk | score vec cols], RU, WuG
            with tc.tile_pool(name="dense", bufs=1) as dp, \
                 tc.tile_pool(name="densepsum", bufs=2, space="PSUM") as pmm, \
                 tc.tile_pool(name="psum1", bufs=1, space="PSUM") as p1, \
                 tc.tile_pool(name="xtp", bufs=1) as xp:

                # bf16 weights: load f32, cast once
                RAf = [dp.tile([128, HID], F32, name=f"RAf{c}") for c in range(2)]
                RUf = [dp.tile([128, HID], F32, name=f"RUf{c}") for c in range(2)]
                RA = [dp.tile([128, HID + 4], BF16, name=f"RA{c}") for c in range(2)]
                RU = [dp.tile([128, HID], BF16, name=f"RU{c}") for c in range(2)]
                WuG = [dp.tile([128, HID], BF16, name=f"WuG{c}") for c in range(2)]
                wgt16 = dp.tile([128, HID], BF16, tag="wgt16")
                nc.vector.tensor_copy(wgt16[:], wgt[:])
                w4s16 = dp.tile([HID, 4], BF16, tag="w4s16")
                nc.vector.tensor_copy(w4s16[:], w4s[:])
                for c in range(2):
                    nc.sync.dma_start(RAf[c][:], W_o[128 * c:128 * (c + 1), :])
                    nc.sync.dma_start(RUf[c][:], W_u[128 * c:128 * (c + 1), :])
                    nc.vector.tensor_copy(RA[c][:, 0:HID], RAf[c][:])
                    nc.vector.tensor_copy(RU[c][:], RUf[c][:])
                # wv = W_o @ w4 (via W_o^T chunks), WuG = W_u @ Wg_top
                for c in range(2):
                    ptp = pmm.tile([128, 128], F32, tag="a")
                    nc.tensor.transpose(ptp[:], RAf[c][:], id_s[:])
                    wt = dp.tile([128, 128], BF16, tag="wt")
                    nc.vector.tensor_copy(wt[:], ptp[:])
                    pwv = pmm.tile([128, 4], F32, tag="b")
                    nc.tensor.matmul(pwv[:], wt[:], w4s16[:], start=True, stop=True)
                    nc.vector.tensor_copy(RA[c][:, HID:HID + 4], pwv[:])

                    ptp2 = pmm.tile([128, 128], F32, tag="a")
                    nc.tensor.transpose(ptp2[:], RUf[c][:], id_s[:])
                    wt2 = dp.tile([128, 128], BF16, tag="wt")
                    nc.vector.tensor_copy(wt2[:], ptp2[:])
                    pwg = pmm.tile([128, 128], F32, tag="b")
                    nc.tensor.matmul(pwg[:], wt2[:], wgt16[:], start=True, stop=True)
                    nc.vector.tensor_copy(WuG[c][:], pwg[:])

                # resident transposed x shard, bf16: 4 chunks of [128, NLP]
                xTc = [xp.tile([128, NLP], BF16, name=f"xTc{c}") for c in range(4)]
                for c in range(4):
                    nc.sync.dma_start(xTc[c][:], xT[128 * c:128 * (c + 1), :].bitcast(BF16))

                xo_s = dp.tile([128, T, HID + 4], F32, tag="xo_s")
                xu_s = dp.tile([128, T, HID], F32, tag="xu_s")
                pdx = p1.tile([2, 128], F32, tag="pdx")
                pcnt = p1.tile([2, 1], F32, tag="pcnt")

                # ---- dense pass A
                for t in range(T):
                    sl = slice(128 * t, 128 * (t + 1))
                    po = pmm.tile([128, HID + 4], F32, tag="a")
                    nc.tensor.matmul(po[:], xTc[0][:, sl], RA[0][:],
                                     start=True, stop=False)
                    nc.tensor.matmul(po[:], xTc[1][:, sl], RA[1][:],
                                     start=False, stop=True)
                    pu = pmm.tile([128, HID], F32, tag="b")
                    nc.tensor.matmul(pu[:], xTc[2][:, sl], RU[0][:],
                                     start=True, stop=False)
                    nc.tensor.matmul(pu[:], xTc[3][:, sl], RU[1][:],
                                     start=False, stop=True)
                    nc.scalar.copy(xo_s[:, t, :], po[:])
                    nc.vector.tensor_copy(xu_s[:, t, :], pu[:])
                    t1t = dp.tile([128, HID], F32, tag="t1t", bufs=3)
                    nc.vector.tensor_scalar_mul(t1t[:], xu_s[:, t, :],
                                                m_s[:, t:t + 1])
                    # deltaX partials: [m, valid-m]^T @ [x_o], and counts
                    nc.tensor.matmul(pdx[:], mm2[:, t, :], xo_s[:, t, 0:HID],
                                     start=(t == 0), stop=(t == T - 1))
                    nc.tensor.matmul(pcnt[:], mm2[:, t, :], ones_col[:],
                                     start=(t == 0), stop=(t == T - 1))
                    # output cols 0:128 = x_o ; cols 128:256 = x_u*m (+= xuhat later)
                    nc.sync.dma_start(outp[sl, 0:HID], xo_s[:, t, 0:HID])
                    nc.sync.dma_start(outp[sl, HID:2 * HID], t1t[:])

                # ---- deltaX AllReduce
                ar_s = dp.tile([2, 132], F32, tag="ar_s")
                nc.vector.memset(ar_s[:], 0.0)
                nc.vector.tensor_copy(ar_s[:, 0:128], pdx[:])
                nc.vector.tensor_copy(ar_s[:, 128:129], pcnt[:])
                nc.sync.dma_start(arin[:], ar_s[:])
                nc.gpsimd.collective_compute(
                    "AllReduce", OP.add, ins=[arin[:]], outs=[arout[:]],
                    replica_groups=RG)
                ars = dp.tile([2, 132], F32, tag="ars")
                nc.sync.dma_start(ars[:], arout[:])
                rec = dp.tile([2, 1], F32, tag="rec")
                nc.vector.reciprocal(rec[:], ars[:, 128:129])
                means = dp.tile([2, 128], F32, tag="means")
                nc.vector.tensor_scalar_mul(means[:], ars[:, 0:128], rec[:])
                ptp3 = pmm.tile([128, 128], F32, tag="a")
                nc.tensor.transpose(ptp3[:, 0:2], means[:], id_s[0:2, 0:2])
                mT = dp.tile([128, 2], F32, tag="mT")
                nc.vector.tensor_copy(mT[:], ptp3[:, 0:2])
                dxc = dp.tile([128, 1], F32, tag="dxc")
                nc.vector.tensor_sub(dxc[:], mT[:, 0:1], mT[:, 1:2])
                pad_ = pmm.tile([1, 128], F32, tag="b")
                nc.tensor.matmul(pad_[:], dxc[:], wd[:], start=True, stop=True)
                adr = dp.tile([1, 128], F32, tag="adr")
                nc.vector.tensor_add(adr[:], pad_[:], bd[:])
                pac = pmm.tile([128, 1], F32, tag="a")
                nc.tensor.matmul(pac[:], adr[:], ones11[:], start=True, stop=True)
                adc = dp.tile([128, 1], F32, tag="adc")
                nc.vector.tensor_copy(adc[:], pac[:])
                pcr = pmm.tile([1, 128], F32, tag="b")
                nc.tensor.matmul(pcr[:], adc[:], wgb[:], start=True, stop=True)
                crow = dp.tile([1, 128], F32, tag="crow")
                nc.vector.tensor_copy(crow[:], pcr[:])

                # ---- dense pass B: message_u (fp8) + scores (f32) -> t0loc rows
                t0st = dp.tile([128, T, TBL_COLS], U8, tag="t0st")
                nc.vector.memset(t0st[:].bitcast(F32), 0.0)
                nc.vector.tensor_copy(
                    t0st[:, :, 128:144].bitcast(F32),
                    xo_s[:, :, HID:HID + 4])
                for t in range(T):
                    sl = slice(128 * t, 128 * (t + 1))
                    pD = pmm.tile([128, HID], F32, tag="a")
                    nc.tensor.matmul(pD[:], xTc[2][:, sl], WuG[0][:],
                                     start=True, stop=False)
                    nc.tensor.matmul(pD[:], xTc[3][:, sl], WuG[1][:],
                                     start=False, stop=False)
                    nc.tensor.matmul(pD[:], ones_row[:], crow[:],
                                     start=False, stop=True)
                    nc.vector.scalar_tensor_tensor(
                        t0st[:, t, 0:HID].bitcast(FP8), pD[:],
                        negm[:, t:t + 1], xu_s[:, t, :],
                        op0=OP.mult, op1=OP.add)

                t0dst = bass.AP(t0loc, 0, [[TBL_COLS, 128], [128 * TBL_COLS, T],
                                           [1, TBL_COLS]])
                nc.sync.dma_start(t0dst, t0st[:])

            # ---------------- AllGather table0; edge layers
            phase = os.environ.get("GNN_PHASE", "full")
            if phase != "dense":
                nc.gpsimd.collective_compute(
                    "AllGather", OP.bypass, ins=[t0loc[0:HROWS, :]],
                    outs=[t0fa[:]], replica_groups=RG)
                nc.gpsimd.collective_compute(
                    "AllGather", OP.bypass, ins=[t0loc[HROWS:NLP, :]],
                    outs=[t0fb[:]], replica_groups=RG)

            if phase not in ("dense", "ag"):
                _edge_layer(nc, tc, 0, s0, (t0fa, t0fb), t0loc, t1loc,
                            idx0_l0, idx1_l0, pb0_l0, pb1_l0, idxd_l0,
                            None, None, outp)

            if phase == "full":
                nc.gpsimd.collective_compute(
                    "AllGather", OP.bypass, ins=[t1loc[0:HROWS, :]],
                    outs=[t1fa[:]], replica_groups=RG)
                nc.gpsimd.collective_compute(
                    "AllGather", OP.bypass, ins=[t1loc[HROWS:NLP, :]],
                    outs=[t1fb[:]], replica_groups=RG)

                _edge_layer(nc, tc, 1, s1, (t1fa, t1fb), t1loc, None,
                            idx0_l1, idx1_l1, pb0_l1, pb1_l1, idxd_l1,
                            scat_l1, oms_l1, outp)

    nc.finalize()
    return nc


def _edge_layer(nc, tc, layer, sched, tfull, tloc, tnext,
                idx0p, idx1p, pb0p, pb1p, idxdp, scatp, omsp, outp):
    from concourse import bass, mybir
    OP = mybir.AluOpType
    AF = mybir.ActivationFunctionType
    F32 = mybir.dt.float32
    I16 = mybir.dt.int16
    U8 = mybir.dt.uint8
    FP8 = mybir.dt.float8e3

    K0, K1 = sched["K0"], sched["K1"]
    coff0, coff1 = sched["coff0"], sched["coff1"]
    C0, C1 = sched["C0"], sched["C1"]
    groups = sched["groups"]
    g0max = max(int(sum(K0[t] for t in g)) for g in groups)
    g1max = max(int(sum(K1[t] for t in g)) for g in groups)

    with tc.tile_pool(name=f"edge{layer}", bufs=1) as ep, \
         tc.tile_pool(name=f"gath{layer}", bufs=2) as gp, \
         tc.tile_pool(name=f"work{layer}", bufs=3) as wp:

        idx0 = ep.tile([128, C0 * 8], I16)
        nc.sync.dma_start(idx0[:], idx0p[:])
        idx1 = ep.tile([128, C1 * 8], I16)
        nc.sync.dma_start(idx1[:], idx1p[:])
        pb0 = ep.tile([128, C0], F32)
        nc.sync.dma_start(pb0[:], pb0p[:])
        pb1 = ep.tile([128, C1], F32)
        nc.sync.dma_start(pb1[:], pb1p[:])
        idxd = ep.tile([128, NLP // 16], I16)
        nc.sync.dma_start(idxd[:], idxdp[:])
        if layer == 1:
            scat = ep.tile([128, NLP // 16], I16)
            nc.sync.dma_start(scat[:], scatp[:])
            oms = ep.tile([128, T], F32)
            nc.sync.dma_start(oms[:], omsp[:])

        # gather all dst rows once: a_tar (+ next-layer scores on layer 0)
        # (dma_gather calls are capped at 1024 indices: larger calls overflow
        #  the Q7-local index scratch and hard-crash the device)
        D = ep.tile([128, T, TBL_COLS], U8)
        for c0 in range(0, T, 8):
            c1 = min(c0 + 8, T)
            ni = (c1 - c0) * 128
            nv = max(0, min(NL - c0 * 128, ni))
            nc.gpsimd.dma_gather(D[:, c0:c1, :], tloc[:],
                                 idxd[:, c0 * 8:c1 * 8], ni, nv, TBL_COLS,
                                 queue_num=_nq())

        if layer == 0:
            msgst = ep.tile([128, T, TBL_COLS], U8)
            nc.vector.memset(msgst[:].bitcast(F32), 0.0)
            # next-layer scores ride along in the D rows: one batched copy
            nc.vector.tensor_copy(msgst[:, :, 128:136].bitcast(F32),
                                  D[:, :, 136:144].bitcast(F32))

        for g in groups:
            gc00, gc01 = int(coff0[g[0]]), int(coff0[g[-1] + 1])
            gc10, gc11 = int(coff1[g[0]]), int(coff1[g[-1] + 1])
            n0, n1 = gc01 - gc00, gc11 - gc10
            G0 = gp.tile([128, g0max, TBL_COLS], U8, tag="G0")
            G1 = gp.tile([128, g1max, TBL_COLS], U8, tag="G1")
            tfa, tfb = tfull
            for cc0 in range(0, n0, 8):
                cc1 = min(cc0 + 8, n0)
                nc.gpsimd.dma_gather(
                    G0[:, cc0:cc1, :], tfa[:],
                    idx0[:, (gc00 + cc0) * 8:(gc00 + cc1) * 8],
                    (cc1 - cc0) * 128, (cc1 - cc0) * 128, TBL_COLS,
                    queue_num=_nq())
            for cc0 in range(0, n1, 8):
                cc1 = min(cc0 + 8, n1)
                nc.gpsimd.dma_gather(
                    G1[:, cc0:cc1, :], tfb[:],
                    idx1[:, (gc10 + cc0) * 8:(gc10 + cc1) * 8],
                    (cc1 - cc0) * 128, (cc1 - cc0) * 128, TBL_COLS,
                    queue_num=_nq())

            if layer == 1:
                scst = gp.tile([128, len(g), HID], F32, tag="scst")

            for ti, t in enumerate(g):
                k0, k1 = int(K0[t]), int(K1[t])
                o0, o1 = int(coff0[t]) - gc00, int(coff1[t]) - gc10
                atar = D[:, t, 132:136].bitcast(F32)
                z0 = wp.tile([128, 1], F32, tag="z0")
                z1 = wp.tile([128, 1], F32, tag="z1")
                ps = []
                for (h, kh, oh, G, pb, co) in (
                        (0, k0, o0, G0, pb0, int(coff0[t])),
                        (1, k1, o1, G1, pb1, int(coff1[t]))):
                    zh = (z0, z1)[h]
                    if kh == 0:
                        nc.vector.memset(zh[:], 0.0)
                        ps.append(None)
                        continue
                    sb = wp.tile([128, kh], F32, tag=f"sb{h}")
                    nc.vector.scalar_tensor_tensor(
                        sb[:], G[:, oh:oh + kh, 128:132].bitcast(F32)[:, :, 0],
                        atar, pb[:, co:co + kh], op0=OP.add, op1=OP.add)
                    nc.vector.scalar_tensor_tensor(
                        sb[:], sb[:], NEG_SLOPE, sb[:], op0=OP.mult, op1=OP.max)
                    ph = wp.tile([128, kh], F32, tag=f"p{h}")
                    nc.scalar.activation(ph[:], sb[:], AF.Exp, accum_out=zh[:])
                    ps.append(ph)
                z = wp.tile([128, 1], F32, tag="z")
                nc.vector.scalar_tensor_tensor(
                    z[:], z0[:], Z_EPS, z1[:], op0=OP.add, op1=OP.add)
                rz = wp.tile([128, 1], F32, tag="rz")
                nc.vector.reciprocal(rz[:], z[:])
                if layer == 1:
                    nc.vector.tensor_mul(rz[:], rz[:], oms[:, t:t + 1])
                # fold 1/Z (and (1-m) on layer 1) into the edge weights
                if ps[0] is not None:
                    nc.vector.tensor_scalar_mul(ps[0][:], ps[0][:], rz[:])
                if ps[1] is not None:
                    nc.vector.tensor_scalar_mul(ps[1][:], ps[1][:], rz[:])

                if layer == 0:
                    dest = msgst[:, t, 0:HID].bitcast(FP8)
                else:
                    dest = scst[:, ti, :]
                chunks = ([(0, k) for k in range(k0)] +
                          [(1, k) for k in range(k1)])
                if not chunks:
                    if layer == 1:
                        nc.vector.memset(dest, 0.0)
                    continue
                # alpha-weighted sum of gathered fp8 msg rows: per-partition
                # multiply-accumulate chain on the vector engine
                nch = len(chunks)
                acc = [wp.tile([128, HID], F32, tag=f"acc{i}", bufs=2,
                               name=f"acc{i}") for i in range(2)]
                for j, (h, k) in enumerate(chunks):
                    G, oh = (G0, o0) if h == 0 else (G1, o1)
                    ph = ps[h]
                    row = G[:, oh + k, 0:HID].bitcast(FP8)
                    out = dest if j == nch - 1 else acc[j % 2][:]
                    if j == 0:
                        nc.vector.tensor_scalar_mul(out, row, ph[:, k:k + 1])
                    else:
                        nc.vector.scalar_tensor_tensor(
                            out, row, ph[:, k:k + 1], acc[(j - 1) % 2][:],
                            op0=OP.mult, op1=OP.add)

            if layer == 1:
                lo = g[0] * 128
                hi = min((g[-1] + 1) * 128, NL)
                nc.gpsimd.dma_scatter_add(
                    outp[:, HID:2 * HID], scst[:, 0:len(g), :],
                    scat[:, lo // 16:(g[-1] + 1) * 128 // 16],
                    len(g) * 128, hi - lo, HID, elem_step=2 * HID,
                    queue_num=_nq())

        if layer == 0:
            tA = HROWS // 128            # 24.5 -> 24 full tiles in half A
            d1 = bass.AP(tnext, 0,
                         [[TBL_COLS, 128], [128 * TBL_COLS, tA], [1, TBL_COLS]])
            nc.sync.dma_start(d1, msgst[:, 0:tA, :])
            d2 = bass.AP(tnext, tA * 128 * TBL_COLS,
                         [[TBL_COLS, 128], [128 * TBL_COLS, 1], [1, TBL_COLS]])
            nc.sync.dma_start(d2, msgst[:, tA:tA + 1, :])
            d3 = bass.AP(tnext, (tA + 1) * 128 * TBL_COLS,
                         [[TBL_COLS, 128], [128 * TBL_COLS, T - tA - 1],
                          [1, TBL_COLS]])
            nc.sync.dma_start(d3, msgst[:, tA + 1:T, :])


# ---------------------------------------------------------------- host driver
def _prep_inputs(x, central_mask, edge_index0, edge_index1,
                 W_o, W_u, w_src0, w_tar0, w_src1, w_tar1, W_diff, b_diff, W_g):
    """Shard + route everything; returns (s0, s1, in_maps)."""
    x = np.asarray(x, np.float32)
    m = np.asarray(central_mask, np.int32)
    e0 = np.asarray(edge_index0, np.int64)
    e1 = np.asarray(edge_index1, np.int64)

    # layer-0 table rows are node-ordered: r0(v) = (v//NL)*NLP + v%NL
    def r0(v):
        return (v // NL) * NLP + (v % NL)

    s0, pc0 = _route_layer(r0(e0[0]), e0[1])

    # layer-1 table rows are in each owner core's layer-0 sorted order
    sp0 = np.concatenate([pc0[c]["sortpos"] for c in range(N_CORES)])

    def r1(v):
        return (v // NL) * NLP + sp0[v]

    s1, pc1 = _route_layer(r1(e1[0]), e1[1])

    w4 = np.stack([np.asarray(w, np.float32) for w in
                   (w_src0, w_tar0, w_src1, w_tar1)], axis=1)
    ident = np.eye(128, dtype=np.float32)

    in_maps = []
    for c in range(N_CORES):
        xs = x[NL * c:NL * (c + 1)]
        xp = np.zeros((NLP, 512), np.float32)
        xp[:NL] = xs
        ms = np.zeros(NLP, np.float32)
        ms[:NL] = m[NL * c:NL * (c + 1)]
        vs = np.zeros(NLP, np.float32)
        vs[:NL] = 1.0
        o1 = pc1[c]["order"]
        omsv = np.zeros(NLP, np.float32)
        omsv[:NL] = 1.0 - ms[o1]
        scat_flat = np.full(NLP, -1, np.int16)
        scat_flat[:NL] = o1.astype(np.int16)
        # layer-1 D-gather reads t1loc rows = layer-0 sorted positions
        idxd1_flat = np.full(NLP, -1, np.int16)
        idxd1_flat[:NL] = pc0[c]["sortpos"][o1].astype(np.int16)

        in_maps.append({
            "xT": np.ascontiguousarray(xp.T).astype(
                ml_dtypes.bfloat16).view(np.int16),
            "m_wrap": np.ascontiguousarray(ms.reshape(T, 128).T),
            "v_wrap": np.ascontiguousarray(vs.reshape(T, 128).T),
            "W_o": np.asarray(W_o, np.float32),
            "W_u": np.asarray(W_u, np.float32),
            "w4": w4,
            "W_diff": np.asarray(W_diff, np.float32),
            "b_diff": np.asarray(b_diff, np.float32).reshape(1, HID),
            "W_g": np.asarray(W_g, np.float32),
            "ident": ident,
            "idx0_l0": pc0[c]["idx0"], "idx1_l0": pc0[c]["idx1"],
            "pb0_l0": pc0[c]["pb0"], "pb1_l0": pc0[c]["pb1"],
            "idxd_l0": _d_idx(pc0[c]["order"]),
            "idx0_l1": pc1[c]["idx0"], "idx1_l1": pc1[c]["idx1"],
            "pb0_l1": pc1[c]["pb0"], "pb1_l1": pc1[c]["pb1"],
            "idxd_l1": _wrap16(idxd1_flat),
            "scat_l1": _wrap16(scat_flat),
            "oms_l1": np.ascontiguousarray(omsv.reshape(T, 128).T),
        })
    return s0, s1, in_maps


_CACHE = {}


def _install_ntff_hook():
    """Register the axon NTFF profiling hook if the image's antenv lacks it."""
    import types
    import contextlib
    try:
        from antenv.axon_hooks import get_axon_ntff_profile_hook  # noqa: F401
        return True
    except ImportError:
        pass
    try:
        if "/root/.axon_site" not in sys.path:
            sys.path.append("/root/.axon_site")
        from trn_agent_boot.trn_boot import _ntff_profile_via_ctypes
        import antenv
        hook = _ntff_profile_via_ctypes("/opt/axon/libaxon_pjrt.so")
        mod = types.ModuleType("antenv.axon_hooks")
        _h = [hook]
        mod.set_axon_ntff_profile_hook = lambda h: _h.__setitem__(0, h)
        mod.get_axon_ntff_profile_hook = lambda: _h[0]
        sys.modules["antenv.axon_hooks"] = mod
        antenv.axon_hooks = mod
        # artifact upload has no bucket in this container; stub it out
        from concourse import bass_utils as _bu
        _bu.upload_artifacts = lambda tmpdir: "local"
        return hook is not None
    except Exception as e:
        print("ntff hook install failed:", e)
        return False


def kernel(**inputs):
    s0, s1, in_maps = _prep_inputs(**inputs)

    from concourse.bass_utils import run_bass_kernel_spmd

    key = (tuple(s0["K0"]), tuple(s0["K1"]), tuple(s1["K0"]), tuple(s1["K1"]))
    if key not in _CACHE:
        _CACHE[key] = build_graph(s0, s1)
    nc = _CACHE[key]

    trace = bool(int(os.environ.get("GNN_TRACE", "0")))
    if trace:
        trace = _install_ntff_hook()
    res = run_bass_kernel_spmd(nc, in_maps, list(range(N_CORES)), trace=trace)
    if trace and res.exec_time_ns is not None:
        print(f"HW exec time: {res.exec_time_ns} ns")
        kernel.last_exec_ns = res.exec_time_ns
        kernel.last_profile = res.profile_json
    out = np.concatenate([res.results[c]["out"][:NL] for c in range(N_CORES)], 0)
    return out.astype(np.float32)


if __name__ == "__main__":
    import reference
    inp = {k: np.asarray(v) for k, v in reference.setup_inputs().items()}
    exp = np.asarray(reference.reference(**inp))
    act = kernel(**inp)
    err = np.abs(act - exp)
    rel = np.linalg.norm(act - exp) / np.linalg.norm(exp)
    print("max abs err:", err.max(), "rel:", rel)

